# revision 26
# baseline (speedup 1.0000x reference)
"""Trainium2 Bass kernel for 8-head MHA (B=2, S=2048, d_model=512).

Sharding: core c -> batch b = c//4, head-pair hp = c%4 (heads 2hp, 2hp+1).
Each core computes q/k/v projections for its 128 out-dims (2 heads), the
masked-softmax attention for those heads, and the output-projection partial
for its 128 in-dims (heads summed on device). Host sums the 4 partials per
batch and adds the output bias.

v2 layout: qd/kd are single [128, S] tiles (head h in partitions 0:64,
h+1 in 64:128); score matmuls use K=64 contraction directly (row-tiled PE,
no operand duplication), so projections evict in one ACT and need no
mirror DMAs. Input DMA issue is spread across four queues (scalar: weights;
sync: xq halves + xv + mask-qh1 + stores; vector: xk halves; gpsimd SWDGE:
mask-qh0) with x tensors split by column half so the first sub-phase's
operands land early. ~24 dummy matmuls at the head keep the PE HAM clock
gate warm through the load phase. Softmax normalization runs off TensorE:
ScalarE ACT-copies the augmented PV psum to a bf16 xt tile inline, then
DVE reciprocal (1-row) + gpsimd partition_broadcast + bf16 DVE multiply,
scheduled one sub-phase late like the baseline. k-half1/q-half1 evictions
ride DVE tensor_scalar so the ScalarE exp stream is never blocked.
"""

import os
import sys
import types
import numpy as np
import ml_dtypes

HEAD = 8
D = 512
DK = 64
B = 2
N_CORES = 8
P = 128

_NC_CACHE = {}
LAST_RESULTS = None  # test harness reads BassKernelResults from here


def _register_ntff_hook():
    """Make run_bass_kernel_spmd(trace=True) work under axon by registering
    the NTFF profile hook that the trimmed antenv package lacks."""
    if "antenv.axon_hooks" in sys.modules:
        return
    try:
        import antenv

        mod = types.ModuleType("antenv.axon_hooks")
        _hook = [None]
        mod.set_axon_ntff_profile_hook = lambda h: _hook.__setitem__(0, h)
        mod.get_axon_ntff_profile_hook = lambda: _hook[0]
        sys.modules["antenv.axon_hooks"] = mod
        antenv.axon_hooks = mod
        if "/root/.axon_site" not in sys.path:
            sys.path.insert(0, "/root/.axon_site")
        from trn_agent_boot.trn_boot import _ntff_profile_via_ctypes

        mod.set_axon_ntff_profile_hook(
            _ntff_profile_via_ctypes("/opt/axon/libaxon_pjrt.so")
        )
    except Exception:
        pass  # tracing degrades; execution still works


def _build_nc(S):
    import concourse.tile as tile
    import concourse.mybir as mybir
    from concourse import bacc
    from concourse.bass import ts
    from contextlib import ExitStack

    from concourse.alu_op_type import AluOpType as Alu

    f32 = mybir.dt.float32
    bf16 = mybir.dt.bfloat16
    AF = mybir.ActivationFunctionType

    KC = S // P       # k chunks (score-tile rows == v s-blocks)
    EC = D // P       # embed chunks for projections
    HW = S // 2       # q half width (1024)
    VG = 130          # v_sb column group: [v_h0(64) | 1 | v_h1(64) | 1]
    NWARM = 24
    debug = os.environ.get("MHA_DEBUG", "0") == "1"

    nc = bacc.Bacc("TRN2", target_bir_lowering=False, debug=False,
                   num_devices=N_CORES)

    xqT = nc.dram_tensor("xqT", [D, S], bf16, kind="ExternalInput").ap()
    xkT = nc.dram_tensor("xkT", [D, S], bf16, kind="ExternalInput").ap()
    xvT = nc.dram_tensor("xvT", [D, S], bf16, kind="ExternalInput").ap()
    maskT = nc.dram_tensor("maskT", [S, S], bf16, kind="ExternalInput").ap()
    wcat = nc.dram_tensor("wcat", [P, 4 * D], bf16, kind="ExternalInput").ap()
    bqk = nc.dram_tensor("bqk", [P, 2], f32, kind="ExternalInput").ap()
    bv_row = nc.dram_tensor("bv_row", [1, P], bf16, kind="ExternalInput").ap()
    outp = nc.dram_tensor("outp", [S, D], bf16, kind="ExternalOutput").ap()
    if debug:
        dbg_qd = nc.dram_tensor("dbg_qd", [P, S], bf16, kind="ExternalOutput").ap()
        dbg_kd = nc.dram_tensor("dbg_kd", [P, S], bf16, kind="ExternalOutput").ap()
        dbg_pt0 = nc.dram_tensor("dbg_pt0", [P, S // 2], bf16, kind="ExternalOutput").ap()
        dbg_pt16 = nc.dram_tensor("dbg_pt16", [P, S // 2], bf16, kind="ExternalOutput").ap()
        dbg_xt0 = nc.dram_tensor("dbg_xt0", [DK + 1, S // 2], f32, kind="ExternalOutput").ap()
        dbg_xhat = nc.dram_tensor("dbg_xhat", [P, S], bf16, kind="ExternalOutput").ap()
        dbg_vsb = nc.dram_tensor("dbg_vsb", [P, (S // P) * VG], bf16, kind="ExternalOutput").ap()

    with tile.TileContext(nc) as tc, ExitStack() as ctx:
        consts = ctx.enter_context(tc.tile_pool(name="consts", bufs=1))
        resid = ctx.enter_context(tc.tile_pool(name="resid", bufs=1))
        mpool = ctx.enter_context(tc.tile_pool(name="maskp", bufs=KC))
        ppool = ctx.enter_context(tc.tile_pool(name="pp", bufs=10))
        xtpool = ctx.enter_context(tc.tile_pool(name="xtp", bufs=2))
        opool = ctx.enter_context(tc.tile_pool(name="outsb", bufs=2))

        ones_row = consts.tile([1, P], bf16)
        nc.vector.memset(ones_row[:], 1.0)
        ones1f = consts.tile([P, DK], f32)  # row DK used as bcast stationary
        nc.vector.memset(ones1f[:], 1.0)
        wsb = consts.tile([P, 4 * D], bf16)   # [wq | wk | wv | wo] chunks
        bqk_sb = consts.tile([P, 2], f32)     # col0 = bq/8, col1 = bk
        bvr_sb = consts.tile([1, P], bf16)
        warm_sb = consts.tile([P, D], bf16)   # warmup matmul operands
        nc.vector.memset(warm_sb[:], 1.0)

        # residents: qd/kd hold head h in partitions 0:64, h+1 in 64:128
        qd = resid.tile([P, S], bf16)
        kd = resid.tile([P, S], bf16)
        v_sb = resid.tile([P, KC * VG], bf16)
        nc.vector.memset(v_sb[:], 1.0)  # pre-set the ones columns
        xhat = resid.tile([P, S], bf16)

        mask_t = [mpool.tile([P, S], bf16, tag="mask", name=f"mask{kc}")
                  for kc in range(KC)]

        # x tiles: xq/xk split by column half for early first-sub-phase start
        xqp = ctx.enter_context(tc.tile_pool(name="xqp", bufs=2 * EC))
        xkp = ctx.enter_context(tc.tile_pool(name="xkp", bufs=2 * EC))
        xvp = ctx.enter_context(tc.tile_pool(name="xvp", bufs=EC))
        xq_t = {}
        xk_t = {}
        for half in range(2):
            for ec in range(EC):
                xq_t[(half, ec)] = xqp.tile([P, HW], bf16, tag="xq", name=f"xq{half}_{ec}")
                xk_t[(half, ec)] = xkp.tile([P, HW], bf16, tag="xk", name=f"xk{half}_{ec}")
        xv_t = [xvp.tile([P, S], bf16, tag="xv", name=f"xv{ec}") for ec in range(EC)]

        # ---- projection section ----
        with tc.tile_pool(name="qk_ps", bufs=2, space="PSUM") as qk_ps, \
             tc.tile_pool(name="warm_ps", bufs=1, space="PSUM") as warm_ps:

            # DMA issue: weights on the scalar queue (tiny, first);
            # xq halves + xv on sync; xk halves on vector; mask qh0 via
            # SWDGE so 16 DMA engines ramp immediately on all fronts
            nc.scalar.dma_start(bqk_sb[:], bqk[:])
            nc.scalar.dma_start(bvr_sb[:], bv_row[:])
            nc.scalar.dma_start(wsb[:], wcat[:])
            for ec in range(EC):
                nc.sync.dma_start(xq_t[(0, ec)][:],
                                  xqT[ec * P:(ec + 1) * P, 0:HW])
            for ec in range(EC):
                nc.scalar.dma_start(xk_t[(0, ec)][:],
                                    xkT[ec * P:(ec + 1) * P, 0:HW])
            for kc in range(KC):
                nc.gpsimd.dma_start(mask_t[kc][:, 0:HW],
                                    maskT[kc * P:(kc + 1) * P, 0:HW])
            for ec in range(EC):
                nc.sync.dma_start(xv_t[ec][:], xvT[ec * P:(ec + 1) * P, :])
            for ec in range(EC):
                nc.scalar.dma_start(xk_t[(1, ec)][:],
                                    xkT[ec * P:(ec + 1) * P, HW:S])
            for ec in range(EC):
                nc.sync.dma_start(xq_t[(1, ec)][:],
                                  xqT[ec * P:(ec + 1) * P, HW:S])

            # PE warmup: keep the HAM clock gate open through the load phase
            wp = warm_ps.tile([P, D], f32)
            for i in range(NWARM):
                nc.tensor.matmul(wp[:, 0:D], warm_sb[:, 0:P],
                                 warm_sb[:, 0:D], start=True, stop=True)

            def proj_matmuls(w0, x_half, half):
                ps = qk_ps.tile([P, HW], f32, tag="qk")
                for ec in range(EC):
                    for st in range(2):
                        nc.tensor.matmul(
                            ps[:, ts(st, 512)], wsb[:, w0 + ec * P: w0 + (ec + 1) * P],
                            x_half[(half, ec)][:, ts(st, 512)],
                            start=(ec == 0), stop=(ec == EC - 1))
                return ps

            # q half0 / k half0 evict on ScalarE (before any exp)
            ps_q0 = proj_matmuls(0, xq_t, 0)
            nc.scalar.activation(qd[:, 0:HW], ps_q0[:], AF.Identity,
                                 bias=bqk_sb[:, 0:1], scale=0.125)
            ps_k0 = proj_matmuls(D, xk_t, 0)
            nc.scalar.activation(kd[:, 0:HW], ps_k0[:], AF.Identity,
                                 bias=bqk_sb[:, 1:2], scale=1.0)

        # ---- attention: flat two-stream pipeline, SKEW-step lag ----
        # (qk_ps/warm_ps are closed: PSUM budget = sc 4 + xaug 2 + misc 2)
        scores_ps = ctx.enter_context(
            tc.tile_pool(name="sc_ps", bufs=2, space="PSUM"))
        xaug_ps = ctx.enter_context(
            tc.tile_pool(name="xa_ps", bufs=1, space="PSUM"))
        misc_ps = ctx.enter_context(
            tc.tile_pool(name="mi_ps", bufs=2, space="PSUM"))

        if True:
            # k/q half1 land mid-attention: run them from the extras stream
            # in 512-col pieces through the misc pool, evicting on DVE so
            # the ScalarE exp stream is never blocked
            def proj_late_unit(w0, x_half, half, dst, st, scale, bias_col):
                def emit():
                    ps = misc_ps.tile([P, 512], f32, tag="mi")
                    for ec in range(EC):
                        nc.tensor.matmul(
                            ps[:], wsb[:, w0 + ec * P: w0 + (ec + 1) * P],
                            x_half[(half, ec)][:, ts(st, 512)],
                            start=(ec == 0), stop=(ec == EC - 1))
                    c0 = half * HW + st * 512
                    if scale is None:
                        nc.vector.tensor_scalar(
                            out=dst[:, c0:c0 + 512], in0=ps[:],
                            scalar1=bqk_sb[:, bias_col:bias_col + 1],
                            scalar2=None, op0=Alu.add)
                    else:
                        nc.vector.tensor_scalar(
                            out=dst[:, c0:c0 + 512], in0=ps[:], scalar1=scale,
                            scalar2=bqk_sb[:, bias_col:bias_col + 1],
                            op0=Alu.mult, op1=Alu.add)
                return emit

            late_proj = [
                proj_late_unit(D, xk_t, 1, kd, 0, None, 1),
                proj_late_unit(D, xk_t, 1, kd, 1, None, 1),
                proj_late_unit(0, xq_t, 1, qd, 0, 0.125, 0),
                proj_late_unit(0, xq_t, 1, qd, 1, 0.125, 0),
            ]

            def vproj_unit(sb):
                def emit():
                    vpt = misc_ps.tile([P, 512], f32, tag="mi")
                    for ec in range(EC):
                        nc.tensor.matmul(vpt[:, 0:P],
                                         xv_t[ec][:, ts(sb, P)],
                                         wsb[:, 2 * D + ec * P: 2 * D + (ec + 1) * P],
                                         start=(ec == 0), stop=False)
                    nc.tensor.matmul(vpt[:, 0:P], ones_row[:], bvr_sb[:],
                                     start=False, stop=True)
                    nc.vector.tensor_copy(v_sb[:, sb * VG: sb * VG + DK],
                                          vpt[:, 0:DK])
                    nc.vector.tensor_copy(
                        v_sb[:, sb * VG + DK + 1: sb * VG + 2 * DK + 1],
                        vpt[:, DK:2 * DK])
                return emit

            def oproj_unit(qb, tail=False):
                def emit():
                    op = misc_ps.tile([P, 512], f32, tag="mi")
                    nc.tensor.matmul(op[:], xhat[:, ts(qb, P)],
                                     wsb[:, 3 * D:4 * D], start=True, stop=True)
                    ob = opool.tile([P, D], bf16, tag="ob")
                    if tail:  # ScalarE is idle after the last exp
                        nc.scalar.copy(ob[:], op[:])
                    else:
                        nc.vector.tensor_copy(ob[:], op[:])
                    nc.sync.dma_start(outp[qb * P:(qb + 1) * P, :], ob[:])
                return emit

            def norm_units(h, qh, xt):
                # denominator row broadcast via K=1 matmul into misc psum,
                # reciprocal in place (partition-base-0, HW-proven), then
                # normalize into xhat -- two 512-col units per sub-phase
                q0 = qh * HW
                units = []
                for q2 in range(2):
                    def emit(q2=q2):
                        bc = misc_ps.tile([DK, 512], f32, tag="mi")
                        nc.tensor.matmul(bc[:], ones1f[DK:DK + 1, :],
                                         xt[DK:DK + 1, ts(q2, 512)],
                                         start=True, stop=True)
                        nc.vector.reciprocal_approx_fast(out=bc[:], in_=bc[:])
                        nc.vector.tensor_mul(
                            xhat[h * DK:(h + 1) * DK,
                                 q0 + q2 * 512: q0 + (q2 + 1) * 512],
                            xt[0:DK, ts(q2, 512)], bc[:])
                    units.append(emit)
                return units

            SPs = [(0, 0), (1, 0), (0, 1), (1, 1)]
            steps = [(spi, h, qh, kc)
                     for spi, (h, qh) in enumerate(SPs) for kc in range(KC)]
            SKEW = 3
            # SP0 extras pop 2/step starting at step 5, so no unit's matmuls
            # sit in the static TensorE queue waiting on late xv/xq-h1 DMAs
            SP0_POP0 = 5
            extras = {0: late_proj[0:2] + [vproj_unit(sb) for sb in range(KC)]
                      + late_proj[2:4],
                      1: [], 2: [], 3: []}
            pts = {}
            xaugs = {}
            tail_norm = []

            def sc_stream(i):
                spi, h, qh, kc = steps[i]
                q0 = qh * HW
                if kc == 0:
                    xaugs[spi] = xaug_ps.tile([DK + 1, HW], f32, tag="xaug", name=f"xa{spi}")
                ex = extras[spi]
                for _ in range(2 if spi == 0 else 1):
                    if ex and (kc >= 2 or (spi == 0 and kc >= SP0_POP0)):
                        ex.pop(0)()
                if spi == 0:
                    # mask qh1 column-halves ride the sync queue behind the
                    # x loads, one issue per early step
                    nc.sync.dma_start(mask_t[kc][:, HW:S],
                                      maskT[kc * P:(kc + 1) * P, HW:S])
                sc = scores_ps.tile([P, HW], f32, tag="sc")
                hs = slice(h * DK, (h + 1) * DK)
                for st in range(2):
                    nc.tensor.matmul(sc[:, ts(st, 512)],
                                     kd[hs, ts(kc, P)],
                                     qd[hs, q0 + st * 512: q0 + (st + 1) * 512],
                                     start=True, stop=True)
                pt = ppool.tile([P, HW], bf16, tag="p")
                nc.scalar.activation(pt[:], sc[:], AF.Exp)
                nc.vector.tensor_mul(pt[:], pt[:], mask_t[kc][:, q0:q0 + HW])
                if debug and i == 0:
                    nc.sync.dma_start(dbg_pt0[:], pt[:])
                if debug and i == 16:
                    nc.sync.dma_start(dbg_pt16[:], pt[:])
                pts[i] = pt

            def pv_stream(j):
                spi, h, qh, kc = steps[j]
                bv0 = (DK + 1) * h
                pt = pts.pop(j)
                for q2 in range(2):
                    nc.tensor.matmul(
                        xaugs[spi][:, ts(q2, 512)],
                        v_sb[:, kc * VG + bv0: kc * VG + bv0 + DK + 1],
                        pt[:, ts(q2, 512)],
                        start=(kc == 0), stop=(kc == KC - 1))
                if kc == KC - 1:
                    # inline evictions free the xaug banks before the next
                    # sub-phase's PV stream touches them: ScalarE ACT-copies
                    # the block, DVE computes 1/denominator from the psum
                    # row; bcast+mult are pushed into the NEXT sub-phase
                    xt = xtpool.tile([DK + 1, HW], f32, tag="xt")
                    nc.scalar.copy(xt[:], xaugs[spi][:])
                    if debug and spi == 0:
                        nc.sync.dma_start(dbg_xt0[:], xt[:])
                    if spi < 3:
                        extras[spi + 1].extend(norm_units(h, qh, xt))
                        if spi == 1:
                            extras[2].extend(oproj_unit(qb) for qb in range(8))
                    else:
                        tail_norm.extend(norm_units(h, qh, xt))

            def pv_allowed(j, i):
                if j > i - SKEW:
                    return False
                if j < KC:
                    # SP0: vproj(kc) is unit 2+kc, popped at SP0_POP0+(2+kc)//2
                    return i >= SP0_POP0 + (2 + j) // 2 + 1
                return True

            pv_next = 0
            i = 0
            while pv_next < len(steps):
                if i < len(steps):
                    sc_stream(i)
                for _ in range(2):  # catch-up: up to two PV steps per loop
                    if pv_next < len(steps) and pv_allowed(pv_next, i):
                        pv_stream(pv_next)
                        pv_next += 1
                i += 1

            # tail: last norm, then the second o-proj batch
            for u in tail_norm:
                u()
            for qb in range(8, 16):
                oproj_unit(qb, tail=True)()

            if debug:
                nc.sync.dma_start(dbg_qd[:], qd[:])
                nc.sync.dma_start(dbg_kd[:], kd[:])
                nc.sync.dma_start(dbg_xhat[:], xhat[:])
                nc.sync.dma_start(dbg_vsb[:], v_sb[:])

    nc.compile()
    return nc


def _get_nc(S):
    if S not in _NC_CACHE:
        _NC_CACHE[S] = _build_nc(S)
    return _NC_CACHE[S]


def kernel(query, key, value, mask, Wq, bq, Wk, bk, Wv, bv, Wo, bo):
    global LAST_RESULTS
    trace = os.environ.get("MHA_TRACE", "0") == "1"
    if trace:
        _register_ntff_hook()

    from concourse.bass_utils import run_bass_kernel_spmd

    query = np.asarray(query)
    key = np.asarray(key)
    value = np.asarray(value)
    mask = np.asarray(mask)
    Wq, bq, Wk, bk = map(np.asarray, (Wq, bq, Wk, bk))
    Wv, bv, Wo, bo = map(np.asarray, (Wv, bv, Wo, bo))

    S = query.shape[1]
    nc = _get_nc(S)

    bf = ml_dtypes.bfloat16
    maskTb = np.ascontiguousarray((mask[0] != 0).T).astype(bf)
    xT = {}
    for b in range(B):
        xT[("q", b)] = np.ascontiguousarray(query[b].T).astype(bf)
        xT[("k", b)] = np.ascontiguousarray(key[b].T).astype(bf)
        xT[("v", b)] = np.ascontiguousarray(value[b].T).astype(bf)

    def w_chunks(Wsl):
        # [128 out, 512 e] -> [128 p(e%128), 512 (ec*128 + out)]
        return np.ascontiguousarray(
            Wsl.T.reshape(4, P, P).transpose(1, 0, 2).reshape(P, D))

    in_maps = []
    for c in range(N_CORES):
        b, hp = divmod(c, 4)
        sl = slice(P * hp, P * hp + P)
        wq_c = w_chunks(Wq[sl, :])
        wk_c = w_chunks(Wk[sl, :])
        wv_c = w_chunks(Wv[sl, :])
        wo_c = np.ascontiguousarray(Wo[:, sl].T)
        wcat = np.concatenate([wq_c, wk_c, wv_c, wo_c], axis=1).astype(bf)
        bqk = np.stack([bq[sl] / 8.0, bk[sl]], axis=1).astype(np.float32)
        in_maps.append({
            "xqT": xT[("q", b)],
            "xkT": xT[("k", b)],
            "xvT": xT[("v", b)],
            "maskT": maskTb,
            "wcat": wcat,
            "bqk": bqk,
            "bv_row": bv[sl].reshape(1, P).astype(bf),
        })

    res = run_bass_kernel_spmd(
        nc, in_maps, core_ids=list(range(N_CORES)),
        trace=trace, trace_cores=[0] if trace else None,
    )
    LAST_RESULTS = res

    out = np.zeros((B, S, D), np.float32)
    for c in range(N_CORES):
        out[c // 4] += res.results[c]["outp"].astype(np.float32)
    out += bo.astype(np.float32)
    return out


# revision 31
# speedup vs baseline: 1.2121x; 1.2121x over previous
"""Trainium2 Bass kernel for 8-head MHA (B=2, S=2048, d_model=512).

Sharding: core c -> batch b = c//4, head-pair hp = c%4 (heads 2hp, 2hp+1).
Each core computes q/k/v projections for its 128 out-dims (2 heads), the
masked-softmax attention for those heads, and the output-projection partial
for its 128 in-dims (heads summed on device). Host sums the 4 partials per
batch and adds the output bias.

v2 layout: qd/kd are single [128, S] tiles (head h in partitions 0:64,
h+1 in 64:128); score matmuls use K=64 contraction directly (row-tiled PE,
no operand duplication), so projections evict in one ACT and need no
mirror DMAs. Input DMA issue is spread across four queues (scalar: weights;
sync: xq halves + xv + mask-qh1 + stores; vector: xk halves; gpsimd SWDGE:
mask-qh0) with x tensors split by column half so the first sub-phase's
operands land early. ~24 dummy matmuls at the head keep the PE HAM clock
gate warm through the load phase. Softmax normalization runs off TensorE:
ScalarE ACT-copies the augmented PV psum to a bf16 xt tile inline, then
DVE reciprocal (1-row) + gpsimd partition_broadcast + bf16 DVE multiply,
scheduled one sub-phase late like the baseline. k-half1/q-half1 evictions
ride DVE tensor_scalar so the ScalarE exp stream is never blocked.
"""

import os
import sys
import types
import numpy as np
import ml_dtypes

HEAD = 8
D = 512
DK = 64
B = 2
N_CORES = 8
P = 128

_NC_CACHE = {}
LAST_RESULTS = None  # test harness reads BassKernelResults from here


def _register_ntff_hook():
    """Make run_bass_kernel_spmd(trace=True) work under axon by registering
    the NTFF profile hook that the trimmed antenv package lacks."""
    if "antenv.axon_hooks" in sys.modules:
        return
    try:
        import antenv

        mod = types.ModuleType("antenv.axon_hooks")
        _hook = [None]
        mod.set_axon_ntff_profile_hook = lambda h: _hook.__setitem__(0, h)
        mod.get_axon_ntff_profile_hook = lambda: _hook[0]
        sys.modules["antenv.axon_hooks"] = mod
        antenv.axon_hooks = mod
        if "/root/.axon_site" not in sys.path:
            sys.path.insert(0, "/root/.axon_site")
        from trn_agent_boot.trn_boot import _ntff_profile_via_ctypes

        mod.set_axon_ntff_profile_hook(
            _ntff_profile_via_ctypes("/opt/axon/libaxon_pjrt.so")
        )
    except Exception:
        pass  # tracing degrades; execution still works


def _build_nc(S):
    import concourse.tile as tile
    import concourse.mybir as mybir
    from concourse import bacc
    from concourse.bass import ts
    from contextlib import ExitStack

    from concourse.alu_op_type import AluOpType as Alu

    f32 = mybir.dt.float32
    bf16 = mybir.dt.bfloat16
    AF = mybir.ActivationFunctionType

    KC = S // P       # k chunks (score-tile rows == v s-blocks)
    EC = D // P       # embed chunks for projections
    HW = S // 2       # q half width (1024)
    VG = 130          # v_sb column group: [v_h0(64) | 1 | v_h1(64) | 1]
    NWARM = 24
    debug = os.environ.get("MHA_DEBUG", "0") == "1"

    nc = bacc.Bacc("TRN2", target_bir_lowering=False, debug=False,
                   num_devices=N_CORES)

    xqT = nc.dram_tensor("xqT", [D, S], bf16, kind="ExternalInput").ap()
    xkT = nc.dram_tensor("xkT", [D, S], bf16, kind="ExternalInput").ap()
    xvT = nc.dram_tensor("xvT", [D, S], bf16, kind="ExternalInput").ap()
    maskT = nc.dram_tensor("maskT", [S, S], bf16, kind="ExternalInput").ap()
    wcat = nc.dram_tensor("wcat", [P, 4 * D], bf16, kind="ExternalInput").ap()
    bqk = nc.dram_tensor("bqk", [P, 2], f32, kind="ExternalInput").ap()
    bv_row = nc.dram_tensor("bv_row", [1, P], bf16, kind="ExternalInput").ap()
    outp = nc.dram_tensor("outp", [S, D], bf16, kind="ExternalOutput").ap()
    if debug:
        dbg_qd = nc.dram_tensor("dbg_qd", [P, S], bf16, kind="ExternalOutput").ap()
        dbg_kd = nc.dram_tensor("dbg_kd", [P, S], bf16, kind="ExternalOutput").ap()
        dbg_pt0 = nc.dram_tensor("dbg_pt0", [P, S // 2], bf16, kind="ExternalOutput").ap()
        dbg_pt16 = nc.dram_tensor("dbg_pt16", [P, S // 2], bf16, kind="ExternalOutput").ap()
        dbg_xt0 = nc.dram_tensor("dbg_xt0", [DK + 1, S // 2], f32, kind="ExternalOutput").ap()
        dbg_xhat = nc.dram_tensor("dbg_xhat", [P, S], bf16, kind="ExternalOutput").ap()
        dbg_vsb = nc.dram_tensor("dbg_vsb", [P, (S // P) * VG], bf16, kind="ExternalOutput").ap()

    with tile.TileContext(nc) as tc, ExitStack() as ctx:
        consts = ctx.enter_context(tc.tile_pool(name="consts", bufs=1))
        resid = ctx.enter_context(tc.tile_pool(name="resid", bufs=1))
        mpool = ctx.enter_context(tc.tile_pool(name="maskp", bufs=KC))
        ppool = ctx.enter_context(tc.tile_pool(name="pp", bufs=10))
        xtpool = ctx.enter_context(tc.tile_pool(name="xtp", bufs=2))
        opool = ctx.enter_context(tc.tile_pool(name="outsb", bufs=2))

        ones_row = consts.tile([1, P], bf16)
        nc.vector.memset(ones_row[:], 1.0)
        ones1f = consts.tile([P, DK], f32)  # row DK used as bcast stationary
        nc.vector.memset(ones1f[:], 1.0)
        wsb = consts.tile([P, 4 * D], bf16)   # [wq | wk | wv | wo] chunks
        bqk_sb = consts.tile([P, 2], f32)     # col0 = bq/8, col1 = bk
        bvr_sb = consts.tile([1, P], bf16)
        warm_sb = consts.tile([P, D], bf16)   # warmup matmul operands
        nc.vector.memset(warm_sb[:], 1.0)

        # residents: per-head duplicated q/k (head slice in BOTH partition
        # halves -> full-array K=128 score matmuls keep the PE HAM warm)
        qd = [resid.tile([P, S], bf16, name=f"qd{h}") for h in range(2)]
        kd = [resid.tile([P, S], bf16, name=f"kd{h}") for h in range(2)]
        v_sb = resid.tile([P, KC * VG], bf16)
        nc.vector.memset(v_sb[:], 1.0)  # pre-set the ones columns
        xhat = resid.tile([P, S], bf16)

        mask_t = [mpool.tile([P, S], bf16, tag="mask", name=f"mask{kc}")
                  for kc in range(KC)]

        # x tiles: xq/xk split by column half for early first-sub-phase start
        xqp = ctx.enter_context(tc.tile_pool(name="xqp", bufs=2 * EC))
        xkp = ctx.enter_context(tc.tile_pool(name="xkp", bufs=2 * EC))
        xvp = ctx.enter_context(tc.tile_pool(name="xvp", bufs=EC))
        xq_t = {}
        xk_t = {}
        for half in range(2):
            for ec in range(EC):
                xq_t[(half, ec)] = xqp.tile([P, HW], bf16, tag="xq", name=f"xq{half}_{ec}")
                xk_t[(half, ec)] = xkp.tile([P, HW], bf16, tag="xk", name=f"xk{half}_{ec}")
        xv_t = [xvp.tile([P, S], bf16, tag="xv", name=f"xv{ec}") for ec in range(EC)]

        # ---- projection section ----
        with tc.tile_pool(name="qk_ps", bufs=2, space="PSUM") as qk_ps, \
             tc.tile_pool(name="warm_ps", bufs=1, space="PSUM") as warm_ps:

            # DMA issue: weights on the scalar queue (tiny, first);
            # xq halves + xv on sync; xk halves on vector; mask qh0 via
            # SWDGE so 16 DMA engines ramp immediately on all fronts
            nc.scalar.dma_start(bqk_sb[:], bqk[:])
            nc.scalar.dma_start(bvr_sb[:], bv_row[:])
            nc.scalar.dma_start(wsb[:], wcat[:])
            for ec in range(EC):
                nc.sync.dma_start(xq_t[(0, ec)][:],
                                  xqT[ec * P:(ec + 1) * P, 0:HW])
            for ec in range(EC):
                nc.scalar.dma_start(xk_t[(0, ec)][:],
                                    xkT[ec * P:(ec + 1) * P, 0:HW])
            for kc in range(KC):
                nc.gpsimd.dma_start(mask_t[kc][:, 0:HW],
                                    maskT[kc * P:(kc + 1) * P, 0:HW])
            for ec in range(EC):
                nc.sync.dma_start(xv_t[ec][:], xvT[ec * P:(ec + 1) * P, :])
            for ec in range(EC):
                nc.scalar.dma_start(xk_t[(1, ec)][:],
                                    xkT[ec * P:(ec + 1) * P, HW:S])
            for ec in range(EC):
                nc.sync.dma_start(xq_t[(1, ec)][:],
                                  xqT[ec * P:(ec + 1) * P, HW:S])

            # PE warmup: keep the HAM clock gate open through the load phase
            wp = warm_ps.tile([P, D], f32)
            for i in range(NWARM):
                nc.tensor.matmul(wp[:, 0:D], warm_sb[:, 0:P],
                                 warm_sb[:, 0:D], start=True, stop=True)

            def proj_matmuls(w0, x_half, half):
                ps = qk_ps.tile([P, HW], f32, tag="qk")
                for ec in range(EC):
                    for st in range(2):
                        nc.tensor.matmul(
                            ps[:, ts(st, 512)], wsb[:, w0 + ec * P: w0 + (ec + 1) * P],
                            x_half[(half, ec)][:, ts(st, 512)],
                            start=(ec == 0), stop=(ec == EC - 1))
                return ps

            # q half0 / k half0 evict on ScalarE (before any exp) into the
            # dup layout: h0 -> partitions 0:64 of dst[0], h1 -> 64:128 of
            # dst[1]; mirror DMAs on the scalar queue fill the other halves
            def evict_half0(dst, ps, bias_col, scale):
                nc.scalar.activation(dst[0][0:DK, 0:HW], ps[0:DK, :],
                                     AF.Identity,
                                     bias=bqk_sb[0:DK, bias_col:bias_col + 1],
                                     scale=scale)
                nc.scalar.activation(dst[1][DK:P, 0:HW], ps[DK:P, :],
                                     AF.Identity,
                                     bias=bqk_sb[DK:P, bias_col:bias_col + 1],
                                     scale=scale)
                nc.scalar.dma_start(dst[0][DK:P, 0:HW], dst[0][0:DK, 0:HW])
                nc.scalar.dma_start(dst[1][0:DK, 0:HW], dst[1][DK:P, 0:HW])

            ps_q0 = proj_matmuls(0, xq_t, 0)
            evict_half0(qd, ps_q0, 0, 0.0625)
            ps_k0 = proj_matmuls(D, xk_t, 0)
            evict_half0(kd, ps_k0, 1, 1.0)

        # ---- attention: flat two-stream pipeline, SKEW-step lag ----
        # (qk_ps/warm_ps are closed: PSUM budget = sc 4 + xaug 2 + misc 2)
        scores_ps = ctx.enter_context(
            tc.tile_pool(name="sc_ps", bufs=2, space="PSUM"))
        xaug_ps = ctx.enter_context(
            tc.tile_pool(name="xa_ps", bufs=1, space="PSUM"))
        misc_ps = ctx.enter_context(
            tc.tile_pool(name="mi_ps", bufs=2, space="PSUM"))

        if True:
            # k/q half1 land mid-attention: run them from the extras stream
            # in 512-col pieces through the misc pool, evicting on DVE so
            # the ScalarE exp stream is never blocked
            def proj_late_unit(w0, x_half, dst, st, scale, bias_col):
                def emit():
                    ps = misc_ps.tile([P, 512], f32, tag="mi")
                    for ec in range(EC):
                        nc.tensor.matmul(
                            ps[:], wsb[:, w0 + ec * P: w0 + (ec + 1) * P],
                            x_half[(1, ec)][:, ts(st, 512)],
                            start=(ec == 0), stop=(ec == EC - 1))
                    c0 = HW + st * 512
                    for h, rows in ((0, slice(0, DK)), (1, slice(DK, P))):
                        if scale is None:
                            nc.vector.tensor_scalar(
                                out=dst[h][rows, c0:c0 + 512], in0=ps[rows, :],
                                scalar1=bqk_sb[rows, bias_col:bias_col + 1],
                                scalar2=None, op0=Alu.add)
                        else:
                            nc.vector.tensor_scalar(
                                out=dst[h][rows, c0:c0 + 512], in0=ps[rows, :],
                                scalar1=scale,
                                scalar2=bqk_sb[rows, bias_col:bias_col + 1],
                                op0=Alu.mult, op1=Alu.add)
                    if st == 1:  # both pieces done: mirror the other halves
                        nc.scalar.dma_start(dst[0][DK:P, HW:S],
                                            dst[0][0:DK, HW:S])
                        nc.scalar.dma_start(dst[1][0:DK, HW:S],
                                            dst[1][DK:P, HW:S])
                return emit

            late_proj = [
                proj_late_unit(D, xk_t, kd, 0, None, 1),
                proj_late_unit(D, xk_t, kd, 1, None, 1),
                proj_late_unit(0, xq_t, qd, 0, 0.0625, 0),
                proj_late_unit(0, xq_t, qd, 1, 0.0625, 0),
            ]

            def vproj_unit(sb):
                def emit():
                    vpt = misc_ps.tile([P, 512], f32, tag="mi")
                    for ec in range(EC):
                        nc.tensor.matmul(vpt[:, 0:P],
                                         xv_t[ec][:, ts(sb, P)],
                                         wsb[:, 2 * D + ec * P: 2 * D + (ec + 1) * P],
                                         start=(ec == 0), stop=False)
                    nc.tensor.matmul(vpt[:, 0:P], ones_row[:], bvr_sb[:],
                                     start=False, stop=True)
                    nc.vector.tensor_copy(v_sb[:, sb * VG: sb * VG + DK],
                                          vpt[:, 0:DK])
                    nc.vector.tensor_copy(
                        v_sb[:, sb * VG + DK + 1: sb * VG + 2 * DK + 1],
                        vpt[:, DK:2 * DK])
                return emit

            def oproj_unit(qb, tail=False):
                def emit():
                    op = misc_ps.tile([P, 512], f32, tag="mi")
                    nc.tensor.matmul(op[:], xhat[:, ts(qb, P)],
                                     wsb[:, 3 * D:4 * D], start=True, stop=True)
                    ob = opool.tile([P, D], bf16, tag="ob")
                    if tail:  # ScalarE is idle after the last exp
                        nc.scalar.copy(ob[:], op[:])
                    else:
                        nc.vector.tensor_copy(ob[:], op[:])
                    nc.sync.dma_start(outp[qb * P:(qb + 1) * P, :], ob[:])
                return emit

            def norm_units(h, qh, xt):
                # denominator row broadcast via K=1 matmul into misc psum,
                # reciprocal in place (partition-base-0, HW-proven), then
                # normalize into xhat -- two 512-col units per sub-phase
                q0 = qh * HW
                units = []
                for q2 in range(2):
                    def emit(q2=q2):
                        bc = misc_ps.tile([DK, 512], f32, tag="mi")
                        nc.tensor.matmul(bc[:], ones1f[DK:DK + 1, :],
                                         xt[DK:DK + 1, ts(q2, 512)],
                                         start=True, stop=True)
                        nc.vector.reciprocal_approx_fast(out=bc[:], in_=bc[:])
                        nc.vector.tensor_mul(
                            xhat[h * DK:(h + 1) * DK,
                                 q0 + q2 * 512: q0 + (q2 + 1) * 512],
                            xt[0:DK, ts(q2, 512)], bc[:])
                    units.append(emit)
                return units

            SPs = [(0, 0), (1, 0), (0, 1), (1, 1)]
            steps = [(spi, h, qh, kc)
                     for spi, (h, qh) in enumerate(SPs) for kc in range(KC)]
            SKEW = 3
            # SP0 extras pop 2/step starting at step 5, so no unit's matmuls
            # sit in the static TensorE queue waiting on late xv/xq-h1 DMAs
            SP0_POP0 = 5
            extras = {0: late_proj[0:2] + [vproj_unit(sb) for sb in range(KC)]
                      + late_proj[2:4],
                      1: [], 2: [], 3: []}
            pts = {}
            xaugs = {}
            tail_norm = []

            def sc_stream(i):
                spi, h, qh, kc = steps[i]
                q0 = qh * HW
                if kc == 0:
                    xaugs[spi] = xaug_ps.tile([DK + 1, HW], f32, tag="xaug", name=f"xa{spi}")
                ex = extras[spi]
                for _ in range(2 if spi == 0 else 1):
                    if ex and (kc >= 2 or (spi == 0 and kc >= SP0_POP0)):
                        ex.pop(0)()
                if spi == 0:
                    # mask qh1 column-halves ride the sync queue behind the
                    # x loads, one issue per early step
                    nc.sync.dma_start(mask_t[kc][:, HW:S],
                                      maskT[kc * P:(kc + 1) * P, HW:S])
                sc = scores_ps.tile([P, HW], f32, tag="sc")
                # K=128 via the duplicated operands: contraction sums the
                # head twice (q pre-scaled by 1/16 cancels it); full-array
                # activity keeps the HAM clock gate open
                for st in range(2):
                    nc.tensor.matmul(sc[:, ts(st, 512)],
                                     kd[h][:, ts(kc, P)],
                                     qd[h][:, q0 + st * 512: q0 + (st + 1) * 512],
                                     start=True, stop=True)
                pt = ppool.tile([P, HW], bf16, tag="p")
                nc.scalar.activation(pt[:], sc[:], AF.Exp)
                nc.vector.tensor_mul(pt[:], pt[:], mask_t[kc][:, q0:q0 + HW])
                if debug and i == 0:
                    nc.sync.dma_start(dbg_pt0[:], pt[:])
                if debug and i == 16:
                    nc.sync.dma_start(dbg_pt16[:], pt[:])
                pts[i] = pt

            def pv_stream(j):
                spi, h, qh, kc = steps[j]
                bv0 = (DK + 1) * h
                pt = pts.pop(j)
                for q2 in range(2):
                    nc.tensor.matmul(
                        xaugs[spi][:, ts(q2, 512)],
                        v_sb[:, kc * VG + bv0: kc * VG + bv0 + DK + 1],
                        pt[:, ts(q2, 512)],
                        start=(kc == 0), stop=(kc == KC - 1))
                if kc == KC - 1:
                    # inline evictions free the xaug banks before the next
                    # sub-phase's PV stream touches them: ScalarE ACT-copies
                    # the block, DVE computes 1/denominator from the psum
                    # row; bcast+mult are pushed into the NEXT sub-phase
                    xt = xtpool.tile([DK + 1, HW], f32, tag="xt")
                    nc.scalar.copy(xt[:], xaugs[spi][:])
                    if debug and spi == 0:
                        nc.sync.dma_start(dbg_xt0[:], xt[:])
                    if spi < 3:
                        extras[spi + 1].extend(norm_units(h, qh, xt))
                        if spi == 1:
                            extras[2].extend(oproj_unit(qb) for qb in range(8))
                    else:
                        tail_norm.extend(norm_units(h, qh, xt))

            def pv_allowed(j, i):
                if j > i - SKEW:
                    return False
                if j < KC:
                    # SP0: vproj(kc) is unit 2+kc, popped at SP0_POP0+(2+kc)//2
                    return i >= SP0_POP0 + (2 + j) // 2 + 1
                return True

            pv_next = 0
            i = 0
            while pv_next < len(steps):
                if i < len(steps):
                    sc_stream(i)
                for _ in range(2):  # catch-up: up to two PV steps per loop
                    if pv_next < len(steps) and pv_allowed(pv_next, i):
                        pv_stream(pv_next)
                        pv_next += 1
                i += 1

            # tail: last norm, then the second o-proj batch
            for u in tail_norm:
                u()
            for qb in range(8, 16):
                oproj_unit(qb, tail=True)()

            if debug:
                nc.sync.dma_start(dbg_qd[:], qd[0][:])
                nc.sync.dma_start(dbg_kd[:], kd[0][:])
                nc.sync.dma_start(dbg_xhat[:], xhat[:])
                nc.sync.dma_start(dbg_vsb[:], v_sb[:])

    nc.compile()
    return nc


def _get_nc(S):
    if S not in _NC_CACHE:
        _NC_CACHE[S] = _build_nc(S)
    return _NC_CACHE[S]


def kernel(query, key, value, mask, Wq, bq, Wk, bk, Wv, bv, Wo, bo):
    global LAST_RESULTS
    trace = os.environ.get("MHA_TRACE", "0") == "1"
    if trace:
        _register_ntff_hook()

    from concourse.bass_utils import run_bass_kernel_spmd

    query = np.asarray(query)
    key = np.asarray(key)
    value = np.asarray(value)
    mask = np.asarray(mask)
    Wq, bq, Wk, bk = map(np.asarray, (Wq, bq, Wk, bk))
    Wv, bv, Wo, bo = map(np.asarray, (Wv, bv, Wo, bo))

    S = query.shape[1]
    nc = _get_nc(S)

    bf = ml_dtypes.bfloat16
    maskTb = np.ascontiguousarray((mask[0] != 0).T).astype(bf)
    xT = {}
    for b in range(B):
        xT[("q", b)] = np.ascontiguousarray(query[b].T).astype(bf)
        xT[("k", b)] = np.ascontiguousarray(key[b].T).astype(bf)
        xT[("v", b)] = np.ascontiguousarray(value[b].T).astype(bf)

    def w_chunks(Wsl):
        # [128 out, 512 e] -> [128 p(e%128), 512 (ec*128 + out)]
        return np.ascontiguousarray(
            Wsl.T.reshape(4, P, P).transpose(1, 0, 2).reshape(P, D))

    in_maps = []
    for c in range(N_CORES):
        b, hp = divmod(c, 4)
        sl = slice(P * hp, P * hp + P)
        wq_c = w_chunks(Wq[sl, :])
        wk_c = w_chunks(Wk[sl, :])
        wv_c = w_chunks(Wv[sl, :])
        wo_c = np.ascontiguousarray(Wo[:, sl].T)
        wcat = np.concatenate([wq_c, wk_c, wv_c, wo_c], axis=1).astype(bf)
        bqk = np.stack([bq[sl] / 16.0, bk[sl]], axis=1).astype(np.float32)
        in_maps.append({
            "xqT": xT[("q", b)],
            "xkT": xT[("k", b)],
            "xvT": xT[("v", b)],
            "maskT": maskTb,
            "wcat": wcat,
            "bqk": bqk,
            "bv_row": bv[sl].reshape(1, P).astype(bf),
        })

    res = run_bass_kernel_spmd(
        nc, in_maps, core_ids=list(range(N_CORES)),
        trace=trace, trace_cores=[0] if trace else None,
    )
    LAST_RESULTS = res

    out = np.zeros((B, S, D), np.float32)
    for c in range(N_CORES):
        out[c // 4] += res.results[c]["outp"].astype(np.float32)
    out += bo.astype(np.float32)
    return out


# revision 34
# speedup vs baseline: 1.3145x; 1.0845x over previous
"""Trainium2 Bass kernel for 8-head MHA (B=2, S=2048, d_model=512).

Sharding: core c -> batch b = c//4, head-pair hp = c%4 (heads 2hp, 2hp+1).
Each core computes q/k/v projections for its 128 out-dims (2 heads), the
masked-softmax attention for those heads, and the output-projection partial
for its 128 in-dims (heads summed on device). Host sums the 4 partials per
batch and adds the output bias.

v2 layout: qd/kd are single [128, S] tiles (head h in partitions 0:64,
h+1 in 64:128); score matmuls use K=64 contraction directly (row-tiled PE,
no operand duplication), so projections evict in one ACT and need no
mirror DMAs. Input DMA issue is spread across four queues (scalar: weights;
sync: xq halves + xv + mask-qh1 + stores; vector: xk halves; gpsimd SWDGE:
mask-qh0) with x tensors split by column half so the first sub-phase's
operands land early. ~24 dummy matmuls at the head keep the PE HAM clock
gate warm through the load phase. Softmax normalization runs off TensorE:
ScalarE ACT-copies the augmented PV psum to a bf16 xt tile inline, then
DVE reciprocal (1-row) + gpsimd partition_broadcast + bf16 DVE multiply,
scheduled one sub-phase late like the baseline. k-half1/q-half1 evictions
ride DVE tensor_scalar so the ScalarE exp stream is never blocked.
"""

import os
import sys
import types
import numpy as np
import ml_dtypes

HEAD = 8
D = 512
DK = 64
B = 2
N_CORES = 8
P = 128

_NC_CACHE = {}
LAST_RESULTS = None  # test harness reads BassKernelResults from here


def _register_ntff_hook():
    """Make run_bass_kernel_spmd(trace=True) work under axon by registering
    the NTFF profile hook that the trimmed antenv package lacks."""
    if "antenv.axon_hooks" in sys.modules:
        return
    try:
        import antenv

        mod = types.ModuleType("antenv.axon_hooks")
        _hook = [None]
        mod.set_axon_ntff_profile_hook = lambda h: _hook.__setitem__(0, h)
        mod.get_axon_ntff_profile_hook = lambda: _hook[0]
        sys.modules["antenv.axon_hooks"] = mod
        antenv.axon_hooks = mod
        if "/root/.axon_site" not in sys.path:
            sys.path.insert(0, "/root/.axon_site")
        from trn_agent_boot.trn_boot import _ntff_profile_via_ctypes

        mod.set_axon_ntff_profile_hook(
            _ntff_profile_via_ctypes("/opt/axon/libaxon_pjrt.so")
        )
    except Exception:
        pass  # tracing degrades; execution still works


def _build_nc(S):
    import concourse.tile as tile
    import concourse.mybir as mybir
    from concourse import bacc
    from concourse.bass import ts
    from contextlib import ExitStack

    from concourse.alu_op_type import AluOpType as Alu

    f32 = mybir.dt.float32
    bf16 = mybir.dt.bfloat16
    AF = mybir.ActivationFunctionType

    KC = S // P       # k chunks (score-tile rows == v s-blocks)
    EC = D // P       # embed chunks for projections
    HW = S // 2       # q half width (1024)
    VG = 130          # v_sb column group: [v_h0(64) | 1 | v_h1(64) | 1]
    NWARM = 24
    debug = os.environ.get("MHA_DEBUG", "0") == "1"

    nc = bacc.Bacc("TRN2", target_bir_lowering=False, debug=False,
                   num_devices=N_CORES)

    xqT = nc.dram_tensor("xqT", [D, S], bf16, kind="ExternalInput").ap()
    xkT = nc.dram_tensor("xkT", [D, S], bf16, kind="ExternalInput").ap()
    xvT = nc.dram_tensor("xvT", [D, S], bf16, kind="ExternalInput").ap()
    maskT = nc.dram_tensor("maskT", [S, S], bf16, kind="ExternalInput").ap()
    wcat = nc.dram_tensor("wcat", [P, 4 * D], bf16, kind="ExternalInput").ap()
    bqk = nc.dram_tensor("bqk", [P, 2], f32, kind="ExternalInput").ap()
    bv_row = nc.dram_tensor("bv_row", [1, P], bf16, kind="ExternalInput").ap()
    outp = nc.dram_tensor("outp", [S, D], bf16, kind="ExternalOutput").ap()
    if debug:
        dbg_qd = nc.dram_tensor("dbg_qd", [P, S], bf16, kind="ExternalOutput").ap()
        dbg_kd = nc.dram_tensor("dbg_kd", [P, S], bf16, kind="ExternalOutput").ap()
        dbg_pt0 = nc.dram_tensor("dbg_pt0", [P, S // 2], bf16, kind="ExternalOutput").ap()
        dbg_pt16 = nc.dram_tensor("dbg_pt16", [P, S // 2], bf16, kind="ExternalOutput").ap()
        dbg_xt0 = nc.dram_tensor("dbg_xt0", [DK + 1, S // 2], f32, kind="ExternalOutput").ap()
        dbg_xhat = nc.dram_tensor("dbg_xhat", [P, S], bf16, kind="ExternalOutput").ap()
        dbg_vsb = nc.dram_tensor("dbg_vsb", [P, (S // P) * VG], bf16, kind="ExternalOutput").ap()

    with tile.TileContext(nc) as tc, ExitStack() as ctx:
        consts = ctx.enter_context(tc.tile_pool(name="consts", bufs=1))
        resid = ctx.enter_context(tc.tile_pool(name="resid", bufs=1))
        mpool = ctx.enter_context(tc.tile_pool(name="maskp", bufs=KC))
        ppool = ctx.enter_context(tc.tile_pool(name="pp", bufs=10))
        xtpool = ctx.enter_context(tc.tile_pool(name="xtp", bufs=2))
        opool = ctx.enter_context(tc.tile_pool(name="outsb", bufs=4))

        ones_row = consts.tile([1, P], bf16)
        nc.vector.memset(ones_row[:], 1.0)
        ones1f = consts.tile([P, DK], f32)  # row DK used as bcast stationary
        nc.vector.memset(ones1f[:], 1.0)
        wsb = consts.tile([P, 4 * D], bf16)   # [wq | wk | wv | wo] chunks
        bqk_sb = consts.tile([P, 2], f32)     # col0 = bq/8, col1 = bk
        bvr_sb = consts.tile([1, P], bf16)
        warm_sb = consts.tile([P, D], bf16)   # warmup matmul operands
        nc.vector.memset(warm_sb[:], 1.0)

        # residents: per-head duplicated q/k (head slice in BOTH partition
        # halves -> full-array K=128 score matmuls keep the PE HAM warm)
        qd = [resid.tile([P, S], bf16, name=f"qd{h}") for h in range(2)]
        kd = [resid.tile([P, S], bf16, name=f"kd{h}") for h in range(2)]
        v_sb = resid.tile([P, KC * VG], bf16)
        nc.vector.memset(v_sb[:], 1.0)  # pre-set the ones columns
        xhat = resid.tile([P, S], bf16)

        mask_t = [mpool.tile([P, S], bf16, tag="mask", name=f"mask{kc}")
                  for kc in range(KC)]

        # x tiles: xq/xk split by column half for early first-sub-phase start
        xqp = ctx.enter_context(tc.tile_pool(name="xqp", bufs=2 * EC))
        xkp = ctx.enter_context(tc.tile_pool(name="xkp", bufs=2 * EC))
        xvp = ctx.enter_context(tc.tile_pool(name="xvp", bufs=EC))
        xq_t = {}
        xk_t = {}
        for half in range(2):
            for ec in range(EC):
                xq_t[(half, ec)] = xqp.tile([P, HW], bf16, tag="xq", name=f"xq{half}_{ec}")
                xk_t[(half, ec)] = xkp.tile([P, HW], bf16, tag="xk", name=f"xk{half}_{ec}")
        xv_t = [xvp.tile([P, S], bf16, tag="xv", name=f"xv{ec}") for ec in range(EC)]

        # ---- projection section ----
        with tc.tile_pool(name="qk_ps", bufs=2, space="PSUM") as qk_ps, \
             tc.tile_pool(name="warm_ps", bufs=1, space="PSUM") as warm_ps:

            # DMA issue: weights on the scalar queue (tiny, first);
            # xq halves + xv on sync; xk halves on vector; mask qh0 via
            # SWDGE so 16 DMA engines ramp immediately on all fronts
            nc.scalar.dma_start(bqk_sb[:], bqk[:])
            nc.scalar.dma_start(bvr_sb[:], bv_row[:])
            nc.scalar.dma_start(wsb[:], wcat[:])
            for ec in range(EC):
                nc.sync.dma_start(xq_t[(0, ec)][:],
                                  xqT[ec * P:(ec + 1) * P, 0:HW])
            for ec in range(EC):
                nc.scalar.dma_start(xk_t[(0, ec)][:],
                                    xkT[ec * P:(ec + 1) * P, 0:HW])
            # mask qh0 rides SWDGE but is gated behind the critical xq/xk
            # half-loads so it cannot flood HBM before attention can start
            mgate = consts.tile([1, 16], bf16)
            nc.gpsimd.tensor_copy(mgate[:], xk_t[(0, EC - 1)][0:1, 0:16])
            for kc in range(KC):
                nc.gpsimd.dma_start(mask_t[kc][:, 0:HW],
                                    maskT[kc * P:(kc + 1) * P, 0:HW])
            for ec in range(EC):
                nc.sync.dma_start(xv_t[ec][:], xvT[ec * P:(ec + 1) * P, :])
            for ec in range(EC):
                nc.scalar.dma_start(xk_t[(1, ec)][:],
                                    xkT[ec * P:(ec + 1) * P, HW:S])
            for ec in range(EC):
                nc.sync.dma_start(xq_t[(1, ec)][:],
                                  xqT[ec * P:(ec + 1) * P, HW:S])

            # PE warmup: keep the HAM clock gate open through the load phase
            wp = warm_ps.tile([P, D], f32)
            for i in range(NWARM):
                nc.tensor.matmul(wp[:, 0:D], warm_sb[:, 0:P],
                                 warm_sb[:, 0:D], start=True, stop=True)

            def proj_matmuls(w0, x_half, half):
                ps = qk_ps.tile([P, HW], f32, tag="qk")
                for ec in range(EC):
                    for st in range(2):
                        nc.tensor.matmul(
                            ps[:, ts(st, 512)], wsb[:, w0 + ec * P: w0 + (ec + 1) * P],
                            x_half[(half, ec)][:, ts(st, 512)],
                            start=(ec == 0), stop=(ec == EC - 1))
                return ps

            # q half0 / k half0 evict on ScalarE (before any exp) into the
            # dup layout: h0 -> partitions 0:64 of dst[0], h1 -> 64:128 of
            # dst[1]; mirror DMAs on the scalar queue fill the other halves
            def evict_half0(dst, ps, bias_col, scale):
                nc.scalar.activation(dst[0][0:DK, 0:HW], ps[0:DK, :],
                                     AF.Identity,
                                     bias=bqk_sb[0:DK, bias_col:bias_col + 1],
                                     scale=scale)
                nc.scalar.activation(dst[1][DK:P, 0:HW], ps[DK:P, :],
                                     AF.Identity,
                                     bias=bqk_sb[DK:P, bias_col:bias_col + 1],
                                     scale=scale)
                nc.scalar.dma_start(dst[0][DK:P, 0:HW], dst[0][0:DK, 0:HW])
                nc.scalar.dma_start(dst[1][0:DK, 0:HW], dst[1][DK:P, 0:HW])

            ps_q0 = proj_matmuls(0, xq_t, 0)
            evict_half0(qd, ps_q0, 0, 0.0625)
            ps_k0 = proj_matmuls(D, xk_t, 0)
            evict_half0(kd, ps_k0, 1, 1.0)

        # ---- attention: flat two-stream pipeline, SKEW-step lag ----
        # (qk_ps/warm_ps are closed: PSUM budget = sc 4 + xaug 2 + misc 2)
        scores_ps = ctx.enter_context(
            tc.tile_pool(name="sc_ps", bufs=2, space="PSUM"))
        xaug_ps = ctx.enter_context(
            tc.tile_pool(name="xa_ps", bufs=1, space="PSUM"))
        misc_ps = ctx.enter_context(
            tc.tile_pool(name="mi_ps", bufs=2, space="PSUM"))

        if True:
            # k/q half1 land mid-attention: run them from the extras stream
            # in 512-col pieces through the misc pool, evicting on DVE so
            # the ScalarE exp stream is never blocked
            def proj_late_unit(w0, x_half, dst, st, scale, bias_col):
                def emit():
                    ps = misc_ps.tile([P, 512], f32, tag="mi")
                    for ec in range(EC):
                        nc.tensor.matmul(
                            ps[:], wsb[:, w0 + ec * P: w0 + (ec + 1) * P],
                            x_half[(1, ec)][:, ts(st, 512)],
                            start=(ec == 0), stop=(ec == EC - 1))
                    c0 = HW + st * 512
                    for h, rows in ((0, slice(0, DK)), (1, slice(DK, P))):
                        if scale is None:
                            nc.vector.tensor_scalar(
                                out=dst[h][rows, c0:c0 + 512], in0=ps[rows, :],
                                scalar1=bqk_sb[rows, bias_col:bias_col + 1],
                                scalar2=None, op0=Alu.add)
                        else:
                            nc.vector.tensor_scalar(
                                out=dst[h][rows, c0:c0 + 512], in0=ps[rows, :],
                                scalar1=scale,
                                scalar2=bqk_sb[rows, bias_col:bias_col + 1],
                                op0=Alu.mult, op1=Alu.add)
                    if st == 1:  # both pieces done: mirror the other halves
                        nc.scalar.dma_start(dst[0][DK:P, HW:S],
                                            dst[0][0:DK, HW:S])
                        nc.scalar.dma_start(dst[1][0:DK, HW:S],
                                            dst[1][DK:P, HW:S])
                return emit

            late_proj = [
                proj_late_unit(D, xk_t, kd, 0, None, 1),
                proj_late_unit(D, xk_t, kd, 1, None, 1),
                proj_late_unit(0, xq_t, qd, 0, 0.0625, 0),
                proj_late_unit(0, xq_t, qd, 1, 0.0625, 0),
            ]

            def vproj_unit(sb):
                def emit():
                    vpt = misc_ps.tile([P, 512], f32, tag="mi")
                    for ec in range(EC):
                        nc.tensor.matmul(vpt[:, 0:P],
                                         xv_t[ec][:, ts(sb, P)],
                                         wsb[:, 2 * D + ec * P: 2 * D + (ec + 1) * P],
                                         start=(ec == 0), stop=False)
                    nc.tensor.matmul(vpt[:, 0:P], ones_row[:], bvr_sb[:],
                                     start=False, stop=True)
                    nc.vector.tensor_copy(v_sb[:, sb * VG: sb * VG + DK],
                                          vpt[:, 0:DK])
                    nc.vector.tensor_copy(
                        v_sb[:, sb * VG + DK + 1: sb * VG + 2 * DK + 1],
                        vpt[:, DK:2 * DK])
                return emit

            def oproj_unit(qb, tail=False):
                def emit():
                    op = misc_ps.tile([P, 512], f32, tag="mi")
                    nc.tensor.matmul(op[:], xhat[:, ts(qb, P)],
                                     wsb[:, 3 * D:4 * D], start=True, stop=True)
                    ob = opool.tile([P, D], bf16, tag="ob")
                    if tail:  # ScalarE is idle after the last exp
                        nc.scalar.copy(ob[:], op[:])
                    else:
                        nc.vector.tensor_copy(ob[:], op[:])
                    nc.sync.dma_start(outp[qb * P:(qb + 1) * P, :], ob[:])
                return emit

            def norm_units(h, qh, xt):
                # denominator row broadcast via K=1 matmul into misc psum,
                # reciprocal in place (partition-base-0, HW-proven), then
                # normalize into xhat -- two 512-col units per sub-phase
                q0 = qh * HW
                units = []
                for q2 in range(2):
                    def emit(q2=q2):
                        bc = misc_ps.tile([DK, 512], f32, tag="mi")
                        nc.tensor.matmul(bc[:], ones1f[DK:DK + 1, :],
                                         xt[DK:DK + 1, ts(q2, 512)],
                                         start=True, stop=True)
                        nc.vector.reciprocal_approx_fast(out=bc[:], in_=bc[:])
                        nc.vector.tensor_mul(
                            xhat[h * DK:(h + 1) * DK,
                                 q0 + q2 * 512: q0 + (q2 + 1) * 512],
                            xt[0:DK, ts(q2, 512)], bc[:])
                    units.append(emit)
                return units

            SPs = [(0, 0), (1, 0), (0, 1), (1, 1)]
            steps = [(spi, h, qh, kc)
                     for spi, (h, qh) in enumerate(SPs) for kc in range(KC)]
            SKEW = 3
            # SP0 extras pop 2/step starting at step 5, so no unit's matmuls
            # sit in the static TensorE queue waiting on late xv/xq-h1 DMAs
            SP0_POP0 = 5
            extras = {0: late_proj[0:2] + [vproj_unit(sb) for sb in range(KC)]
                      + late_proj[2:4],
                      1: [], 2: [], 3: []}
            pts = {}
            xaugs = {}
            tail_norm = []

            def sc_stream(i):
                spi, h, qh, kc = steps[i]
                q0 = qh * HW
                if kc == 0:
                    xaugs[spi] = xaug_ps.tile([DK + 1, HW], f32, tag="xaug", name=f"xa{spi}")
                ex = extras[spi]
                for _ in range(2 if spi == 0 else 1):
                    if ex and (kc >= 2 or (spi == 0 and kc >= SP0_POP0)):
                        ex.pop(0)()
                if spi == 0:
                    # mask qh1 column-halves ride the sync queue behind the
                    # x loads, one issue per early step
                    nc.sync.dma_start(mask_t[kc][:, HW:S],
                                      maskT[kc * P:(kc + 1) * P, HW:S])
                sc = scores_ps.tile([P, HW], f32, tag="sc")
                # K=128 via the duplicated operands: contraction sums the
                # head twice (q pre-scaled by 1/16 cancels it); full-array
                # activity keeps the HAM clock gate open
                for st in range(2):
                    nc.tensor.matmul(sc[:, ts(st, 512)],
                                     kd[h][:, ts(kc, P)],
                                     qd[h][:, q0 + st * 512: q0 + (st + 1) * 512],
                                     start=True, stop=True)
                pt = ppool.tile([P, HW], bf16, tag="p")
                nc.scalar.activation(pt[:], sc[:], AF.Exp)
                nc.vector.tensor_mul(pt[:], pt[:], mask_t[kc][:, q0:q0 + HW])
                if debug and i == 0:
                    nc.sync.dma_start(dbg_pt0[:], pt[:])
                if debug and i == 16:
                    nc.sync.dma_start(dbg_pt16[:], pt[:])
                pts[i] = pt

            def pv_stream(j):
                spi, h, qh, kc = steps[j]
                bv0 = (DK + 1) * h
                pt = pts.pop(j)
                for q2 in range(2):
                    nc.tensor.matmul(
                        xaugs[spi][:, ts(q2, 512)],
                        v_sb[:, kc * VG + bv0: kc * VG + bv0 + DK + 1],
                        pt[:, ts(q2, 512)],
                        start=(kc == 0), stop=(kc == KC - 1))
                if kc == KC - 1:
                    # inline evictions free the xaug banks before the next
                    # sub-phase's PV stream touches them: ScalarE ACT-copies
                    # the block, DVE computes 1/denominator from the psum
                    # row; bcast+mult are pushed into the NEXT sub-phase
                    xt = xtpool.tile([DK + 1, HW], f32, tag="xt")
                    nc.scalar.copy(xt[:], xaugs[spi][:])
                    if debug and spi == 0:
                        nc.sync.dma_start(dbg_xt0[:], xt[:])
                    if spi < 3:
                        extras[spi + 1].extend(norm_units(h, qh, xt))
                        if spi == 1:
                            extras[2].extend(oproj_unit(qb) for qb in range(8))
                    else:
                        tail_norm.extend(norm_units(h, qh, xt))

            def pv_allowed(j, i):
                if j > i - SKEW:
                    return False
                if j < KC:
                    # SP0: vproj(kc) is unit 2+kc, popped at SP0_POP0+(2+kc)//2
                    return i >= SP0_POP0 + (2 + j) // 2 + 1
                return True

            pv_next = 0
            i = 0
            while pv_next < len(steps):
                if i < len(steps):
                    sc_stream(i)
                for _ in range(2):  # catch-up: up to two PV steps per loop
                    if pv_next < len(steps) and pv_allowed(pv_next, i):
                        pv_stream(pv_next)
                        pv_next += 1
                i += 1

            # tail: last norm, then the second o-proj batch
            for u in tail_norm:
                u()
            for qb in range(8, 16):
                oproj_unit(qb, tail=(qb % 2 == 0))()

            if debug:
                nc.sync.dma_start(dbg_qd[:], qd[0][:])
                nc.sync.dma_start(dbg_kd[:], kd[0][:])
                nc.sync.dma_start(dbg_xhat[:], xhat[:])
                nc.sync.dma_start(dbg_vsb[:], v_sb[:])

    nc.compile()
    return nc


def _get_nc(S):
    if S not in _NC_CACHE:
        _NC_CACHE[S] = _build_nc(S)
    return _NC_CACHE[S]


def kernel(query, key, value, mask, Wq, bq, Wk, bk, Wv, bv, Wo, bo):
    global LAST_RESULTS
    trace = os.environ.get("MHA_TRACE", "0") == "1"
    if trace:
        _register_ntff_hook()

    from concourse.bass_utils import run_bass_kernel_spmd

    query = np.asarray(query)
    key = np.asarray(key)
    value = np.asarray(value)
    mask = np.asarray(mask)
    Wq, bq, Wk, bk = map(np.asarray, (Wq, bq, Wk, bk))
    Wv, bv, Wo, bo = map(np.asarray, (Wv, bv, Wo, bo))

    S = query.shape[1]
    nc = _get_nc(S)

    bf = ml_dtypes.bfloat16
    maskTb = np.ascontiguousarray((mask[0] != 0).T).astype(bf)
    xT = {}
    for b in range(B):
        xT[("q", b)] = np.ascontiguousarray(query[b].T).astype(bf)
        xT[("k", b)] = np.ascontiguousarray(key[b].T).astype(bf)
        xT[("v", b)] = np.ascontiguousarray(value[b].T).astype(bf)

    def w_chunks(Wsl):
        # [128 out, 512 e] -> [128 p(e%128), 512 (ec*128 + out)]
        return np.ascontiguousarray(
            Wsl.T.reshape(4, P, P).transpose(1, 0, 2).reshape(P, D))

    in_maps = []
    for c in range(N_CORES):
        b, hp = divmod(c, 4)
        sl = slice(P * hp, P * hp + P)
        wq_c = w_chunks(Wq[sl, :])
        wk_c = w_chunks(Wk[sl, :])
        wv_c = w_chunks(Wv[sl, :])
        wo_c = np.ascontiguousarray(Wo[:, sl].T)
        wcat = np.concatenate([wq_c, wk_c, wv_c, wo_c], axis=1).astype(bf)
        bqk = np.stack([bq[sl] / 16.0, bk[sl]], axis=1).astype(np.float32)
        in_maps.append({
            "xqT": xT[("q", b)],
            "xkT": xT[("k", b)],
            "xvT": xT[("v", b)],
            "maskT": maskTb,
            "wcat": wcat,
            "bqk": bqk,
            "bv_row": bv[sl].reshape(1, P).astype(bf),
        })

    res = run_bass_kernel_spmd(
        nc, in_maps, core_ids=list(range(N_CORES)),
        trace=trace, trace_cores=[0] if trace else None,
    )
    LAST_RESULTS = res

    out = np.zeros((B, S, D), np.float32)
    for c in range(N_CORES):
        out[c // 4] += res.results[c]["outp"].astype(np.float32)
    out += bo.astype(np.float32)
    return out


# revision 36
# speedup vs baseline: 1.3200x; 1.0042x over previous
"""Trainium2 Bass kernel for 8-head MHA (B=2, S=2048, d_model=512).

Sharding: core c -> batch b = c//4, head-pair hp = c%4 (heads 2hp, 2hp+1).
Each core computes q/k/v projections for its 128 out-dims (2 heads), the
masked-softmax attention for those heads, and the output-projection partial
for its 128 in-dims (heads summed on device). Host sums the 4 partials per
batch and adds the output bias.

Score matmuls use the duplicated-head K=128 form (head slice in both
partition halves of qd/kd, q pre-scaled by 1/16): full-array activity is
required to hold the PE HAM clock gate open -- K=64 row-tiled scores
measured ~1.8x slower per matmul because the array throttles to 1.2 GHz.

Head phase: only the first sub-phase's operands (fused wq|wk|wv|wo +
biases + xk half-0 on the scalar HWDGE queue, xq half-0 on sync) are
issued immediately; xk half-1, xv, xq half-1 and the qh0 mask tiles ride
the gpsimd SWDGE queue gated behind the xk-h0 load, because the 16 DMA
engines progress all queued transfers concurrently and ungated bulk
loads starve the critical path. ~24 dummy matmuls bridge the load phase
to keep the HAM warm. k/q half-1 projections run from the extras stream
through the misc psum pool with DVE tensor_scalar evictions so the
ScalarE exp stream is never blocked; the PV stream uses a catch-up
schedule gated on when each vproj unit was emitted so no matmul sits in
the static TensorE queue waiting on late xv/xq-h1 DMAs. PV eviction is
an inline ScalarE ACT-copy to an f32 xt tile (frees the single xaug psum
buffer before the next sub-phase's PV writes); normalization (K=1
ones-matmul broadcast + in-place DVE reciprocal + multiply) is deferred
one sub-phase as in the baseline. Mask qh1 column-halves and output
stores ride the sync queue during attention.
"""

import os
import sys
import types
import numpy as np
import ml_dtypes

HEAD = 8
D = 512
DK = 64
B = 2
N_CORES = 8
P = 128

_NC_CACHE = {}
LAST_RESULTS = None  # test harness reads BassKernelResults from here


def _register_ntff_hook():
    """Make run_bass_kernel_spmd(trace=True) work under axon by registering
    the NTFF profile hook that the trimmed antenv package lacks."""
    if "antenv.axon_hooks" in sys.modules:
        return
    try:
        import antenv

        mod = types.ModuleType("antenv.axon_hooks")
        _hook = [None]
        mod.set_axon_ntff_profile_hook = lambda h: _hook.__setitem__(0, h)
        mod.get_axon_ntff_profile_hook = lambda: _hook[0]
        sys.modules["antenv.axon_hooks"] = mod
        antenv.axon_hooks = mod
        if "/root/.axon_site" not in sys.path:
            sys.path.insert(0, "/root/.axon_site")
        from trn_agent_boot.trn_boot import _ntff_profile_via_ctypes

        mod.set_axon_ntff_profile_hook(
            _ntff_profile_via_ctypes("/opt/axon/libaxon_pjrt.so")
        )
    except Exception:
        pass  # tracing degrades; execution still works


def _build_nc(S):
    import concourse.tile as tile
    import concourse.mybir as mybir
    from concourse import bacc
    from concourse.bass import ts
    from contextlib import ExitStack

    from concourse.alu_op_type import AluOpType as Alu

    f32 = mybir.dt.float32
    bf16 = mybir.dt.bfloat16
    AF = mybir.ActivationFunctionType

    KC = S // P       # k chunks (score-tile rows == v s-blocks)
    EC = D // P       # embed chunks for projections
    HW = S // 2       # q half width (1024)
    VG = 130          # v_sb column group: [v_h0(64) | 1 | v_h1(64) | 1]
    NWARM = 24
    debug = os.environ.get("MHA_DEBUG", "0") == "1"

    nc = bacc.Bacc("TRN2", target_bir_lowering=False, debug=False,
                   num_devices=N_CORES)

    xqT = nc.dram_tensor("xqT", [D, S], bf16, kind="ExternalInput").ap()
    xkT = nc.dram_tensor("xkT", [D, S], bf16, kind="ExternalInput").ap()
    xvT = nc.dram_tensor("xvT", [D, S], bf16, kind="ExternalInput").ap()
    maskT = nc.dram_tensor("maskT", [S, S], bf16, kind="ExternalInput").ap()
    wcat = nc.dram_tensor("wcat", [P, 4 * D], bf16, kind="ExternalInput").ap()
    bqk = nc.dram_tensor("bqk", [P, 2], f32, kind="ExternalInput").ap()
    bv_row = nc.dram_tensor("bv_row", [1, P], bf16, kind="ExternalInput").ap()
    outp = nc.dram_tensor("outp", [S, D], bf16, kind="ExternalOutput").ap()
    if debug:
        dbg_qd = nc.dram_tensor("dbg_qd", [P, S], bf16, kind="ExternalOutput").ap()
        dbg_kd = nc.dram_tensor("dbg_kd", [P, S], bf16, kind="ExternalOutput").ap()
        dbg_pt0 = nc.dram_tensor("dbg_pt0", [P, S // 2], bf16, kind="ExternalOutput").ap()
        dbg_pt16 = nc.dram_tensor("dbg_pt16", [P, S // 2], bf16, kind="ExternalOutput").ap()
        dbg_xt0 = nc.dram_tensor("dbg_xt0", [DK + 1, S // 2], f32, kind="ExternalOutput").ap()
        dbg_xhat = nc.dram_tensor("dbg_xhat", [P, S], bf16, kind="ExternalOutput").ap()
        dbg_vsb = nc.dram_tensor("dbg_vsb", [P, (S // P) * VG], bf16, kind="ExternalOutput").ap()

    with tile.TileContext(nc) as tc, ExitStack() as ctx:
        consts = ctx.enter_context(tc.tile_pool(name="consts", bufs=1))
        resid = ctx.enter_context(tc.tile_pool(name="resid", bufs=1))
        mpool = ctx.enter_context(tc.tile_pool(name="maskp", bufs=KC))
        ppool = ctx.enter_context(tc.tile_pool(name="pp", bufs=10))
        xtpool = ctx.enter_context(tc.tile_pool(name="xtp", bufs=2))
        opool = ctx.enter_context(tc.tile_pool(name="outsb", bufs=4))

        ones_row = consts.tile([1, P], bf16)
        nc.vector.memset(ones_row[:], 1.0)
        ones1f = consts.tile([P, DK], f32)  # row DK used as bcast stationary
        nc.vector.memset(ones1f[:], 1.0)
        wsb = consts.tile([P, 4 * D], bf16)   # [wq | wk | wv | wo] chunks
        bqk_sb = consts.tile([P, 2], f32)     # col0 = bq/8, col1 = bk
        bvr_sb = consts.tile([1, P], bf16)
        warm_sb = consts.tile([P, D], bf16)   # warmup matmul operands
        nc.vector.memset(warm_sb[:], 1.0)

        # residents: per-head duplicated q/k (head slice in BOTH partition
        # halves -> full-array K=128 score matmuls keep the PE HAM warm)
        qd = [resid.tile([P, S], bf16, name=f"qd{h}") for h in range(2)]
        kd = [resid.tile([P, S], bf16, name=f"kd{h}") for h in range(2)]
        v_sb = resid.tile([P, KC * VG], bf16)
        nc.vector.memset(v_sb[:], 1.0)  # pre-set the ones columns
        xhat = resid.tile([P, S], bf16)

        mask_t = [mpool.tile([P, S], bf16, tag="mask", name=f"mask{kc}")
                  for kc in range(KC)]

        # x tiles: xq/xk split by column half for early first-sub-phase start
        xqp = ctx.enter_context(tc.tile_pool(name="xqp", bufs=2 * EC))
        xkp = ctx.enter_context(tc.tile_pool(name="xkp", bufs=2 * EC))
        xvp = ctx.enter_context(tc.tile_pool(name="xvp", bufs=EC))
        xq_t = {}
        xk_t = {}
        for half in range(2):
            for ec in range(EC):
                xq_t[(half, ec)] = xqp.tile([P, HW], bf16, tag="xq", name=f"xq{half}_{ec}")
                xk_t[(half, ec)] = xkp.tile([P, HW], bf16, tag="xk", name=f"xk{half}_{ec}")
        xv_t = [xvp.tile([P, S], bf16, tag="xv", name=f"xv{ec}") for ec in range(EC)]

        # ---- projection section ----
        with tc.tile_pool(name="qk_ps", bufs=2, space="PSUM") as qk_ps, \
             tc.tile_pool(name="warm_ps", bufs=1, space="PSUM") as warm_ps:

            # DMA issue: weights on the scalar queue (tiny, first);
            # xq halves + xv on sync; xk halves on vector; mask qh0 via
            # SWDGE so 16 DMA engines ramp immediately on all fronts
            nc.scalar.dma_start(bqk_sb[:], bqk[:])
            nc.scalar.dma_start(bvr_sb[:], bv_row[:])
            nc.scalar.dma_start(wsb[:], wcat[:])
            for ec in range(EC):
                nc.sync.dma_start(xq_t[(0, ec)][:],
                                  xqT[ec * P:(ec + 1) * P, 0:HW])
            for ec in range(EC):
                nc.scalar.dma_start(xk_t[(0, ec)][:],
                                    xkT[ec * P:(ec + 1) * P, 0:HW])
            # everything that is not needed for the first sub-phase rides
            # SWDGE gated behind the critical xq/xk half-loads, so those get
            # the full HBM bandwidth (the 16 DMA engines otherwise progress
            # all queued transfers concurrently)
            mgate = consts.tile([1, 16], bf16)
            nc.gpsimd.tensor_copy(mgate[:], xk_t[(0, EC - 1)][0:1, 0:16])
            for ec in range(EC):
                nc.gpsimd.dma_start(xk_t[(1, ec)][:],
                                    xkT[ec * P:(ec + 1) * P, HW:S])
            for ec in range(EC):
                nc.gpsimd.dma_start(xv_t[ec][:], xvT[ec * P:(ec + 1) * P, :])
            for ec in range(EC):
                nc.gpsimd.dma_start(xq_t[(1, ec)][:],
                                    xqT[ec * P:(ec + 1) * P, HW:S])
            for kc in range(KC):
                nc.gpsimd.dma_start(mask_t[kc][:, 0:HW],
                                    maskT[kc * P:(kc + 1) * P, 0:HW])

            # PE warmup: keep the HAM clock gate open through the load phase
            wp = warm_ps.tile([P, D], f32)
            for i in range(NWARM):
                nc.tensor.matmul(wp[:, 0:D], warm_sb[:, 0:P],
                                 warm_sb[:, 0:D], start=True, stop=True)

            def proj_matmuls(w0, x_half, half):
                ps = qk_ps.tile([P, HW], f32, tag="qk")
                for ec in range(EC):
                    for st in range(2):
                        nc.tensor.matmul(
                            ps[:, ts(st, 512)], wsb[:, w0 + ec * P: w0 + (ec + 1) * P],
                            x_half[(half, ec)][:, ts(st, 512)],
                            start=(ec == 0), stop=(ec == EC - 1))
                return ps

            # q half0 / k half0 evict on ScalarE (before any exp) into the
            # dup layout: h0 -> partitions 0:64 of dst[0], h1 -> 64:128 of
            # dst[1]; mirror DMAs on the scalar queue fill the other halves
            def evict_half0(dst, ps, bias_col, scale):
                nc.scalar.activation(dst[0][0:DK, 0:HW], ps[0:DK, :],
                                     AF.Identity,
                                     bias=bqk_sb[0:DK, bias_col:bias_col + 1],
                                     scale=scale)
                nc.scalar.activation(dst[1][DK:P, 0:HW], ps[DK:P, :],
                                     AF.Identity,
                                     bias=bqk_sb[DK:P, bias_col:bias_col + 1],
                                     scale=scale)
                nc.scalar.dma_start(dst[0][DK:P, 0:HW], dst[0][0:DK, 0:HW])
                nc.scalar.dma_start(dst[1][0:DK, 0:HW], dst[1][DK:P, 0:HW])

            ps_q0 = proj_matmuls(0, xq_t, 0)
            evict_half0(qd, ps_q0, 0, 0.0625)
            ps_k0 = proj_matmuls(D, xk_t, 0)
            evict_half0(kd, ps_k0, 1, 1.0)

        # ---- attention: flat two-stream pipeline, SKEW-step lag ----
        # (qk_ps/warm_ps are closed: PSUM budget = sc 4 + xaug 2 + misc 2)
        scores_ps = ctx.enter_context(
            tc.tile_pool(name="sc_ps", bufs=2, space="PSUM"))
        xaug_ps = ctx.enter_context(
            tc.tile_pool(name="xa_ps", bufs=1, space="PSUM"))
        misc_ps = ctx.enter_context(
            tc.tile_pool(name="mi_ps", bufs=2, space="PSUM"))

        if True:
            # k/q half1 land mid-attention: run them from the extras stream
            # in 512-col pieces through the misc pool, evicting on DVE so
            # the ScalarE exp stream is never blocked
            def proj_late_unit(w0, x_half, dst, st, scale, bias_col):
                def emit():
                    ps = misc_ps.tile([P, 512], f32, tag="mi")
                    for ec in range(EC):
                        nc.tensor.matmul(
                            ps[:], wsb[:, w0 + ec * P: w0 + (ec + 1) * P],
                            x_half[(1, ec)][:, ts(st, 512)],
                            start=(ec == 0), stop=(ec == EC - 1))
                    c0 = HW + st * 512
                    for h, rows in ((0, slice(0, DK)), (1, slice(DK, P))):
                        if scale is None:
                            nc.vector.tensor_scalar(
                                out=dst[h][rows, c0:c0 + 512], in0=ps[rows, :],
                                scalar1=bqk_sb[rows, bias_col:bias_col + 1],
                                scalar2=None, op0=Alu.add)
                        else:
                            nc.vector.tensor_scalar(
                                out=dst[h][rows, c0:c0 + 512], in0=ps[rows, :],
                                scalar1=scale,
                                scalar2=bqk_sb[rows, bias_col:bias_col + 1],
                                op0=Alu.mult, op1=Alu.add)
                    if st == 1:  # both pieces done: mirror the other halves
                        nc.scalar.dma_start(dst[0][DK:P, HW:S],
                                            dst[0][0:DK, HW:S])
                        nc.scalar.dma_start(dst[1][0:DK, HW:S],
                                            dst[1][DK:P, HW:S])
                return emit

            late_proj = [
                proj_late_unit(D, xk_t, kd, 0, None, 1),
                proj_late_unit(D, xk_t, kd, 1, None, 1),
                proj_late_unit(0, xq_t, qd, 0, 0.0625, 0),
                proj_late_unit(0, xq_t, qd, 1, 0.0625, 0),
            ]

            def vproj_unit(sb):
                def emit():
                    vpt = misc_ps.tile([P, 512], f32, tag="mi")
                    for ec in range(EC):
                        nc.tensor.matmul(vpt[:, 0:P],
                                         xv_t[ec][:, ts(sb, P)],
                                         wsb[:, 2 * D + ec * P: 2 * D + (ec + 1) * P],
                                         start=(ec == 0), stop=False)
                    nc.tensor.matmul(vpt[:, 0:P], ones_row[:], bvr_sb[:],
                                     start=False, stop=True)
                    nc.vector.tensor_copy(v_sb[:, sb * VG: sb * VG + DK],
                                          vpt[:, 0:DK])
                    nc.vector.tensor_copy(
                        v_sb[:, sb * VG + DK + 1: sb * VG + 2 * DK + 1],
                        vpt[:, DK:2 * DK])
                return emit

            def oproj_unit(qb, tail=False):
                def emit():
                    op = misc_ps.tile([P, 512], f32, tag="mi")
                    nc.tensor.matmul(op[:], xhat[:, ts(qb, P)],
                                     wsb[:, 3 * D:4 * D], start=True, stop=True)
                    ob = opool.tile([P, D], bf16, tag="ob")
                    if tail:  # ScalarE is idle after the last exp
                        nc.scalar.copy(ob[:], op[:])
                    else:
                        nc.vector.tensor_copy(ob[:], op[:])
                    nc.sync.dma_start(outp[qb * P:(qb + 1) * P, :], ob[:])
                return emit

            def norm_units(h, qh, xt):
                # denominator row broadcast via K=1 matmul into misc psum,
                # reciprocal in place (partition-base-0, HW-proven), then
                # normalize into xhat -- two 512-col units per sub-phase
                q0 = qh * HW
                units = []
                for q2 in range(2):
                    def emit(q2=q2):
                        bc = misc_ps.tile([DK, 512], f32, tag="mi")
                        nc.tensor.matmul(bc[:], ones1f[DK:DK + 1, :],
                                         xt[DK:DK + 1, ts(q2, 512)],
                                         start=True, stop=True)
                        nc.vector.reciprocal_approx_fast(out=bc[:], in_=bc[:])
                        nc.vector.tensor_mul(
                            xhat[h * DK:(h + 1) * DK,
                                 q0 + q2 * 512: q0 + (q2 + 1) * 512],
                            xt[0:DK, ts(q2, 512)], bc[:])
                    units.append(emit)
                return units

            SPs = [(0, 0), (1, 0), (0, 1), (1, 1)]
            steps = [(spi, h, qh, kc)
                     for spi, (h, qh) in enumerate(SPs) for kc in range(KC)]
            SKEW = 3
            # SP0 extras pop 2/step starting at step 5, so no unit's matmuls
            # sit in the static TensorE queue waiting on late xv/xq-h1 DMAs
            SP0_POP0 = 5
            extras = {0: late_proj[0:2] + [vproj_unit(sb) for sb in range(KC)]
                      + late_proj[2:4],
                      1: [], 2: [], 3: []}
            pts = {}
            xaugs = {}
            tail_norm = []

            def sc_stream(i):
                spi, h, qh, kc = steps[i]
                q0 = qh * HW
                if kc == 0:
                    xaugs[spi] = xaug_ps.tile([DK + 1, HW], f32, tag="xaug", name=f"xa{spi}")
                ex = extras[spi]
                for _ in range(2 if spi == 0 else 1):
                    if ex and (kc >= 2 or (spi == 0 and kc >= SP0_POP0)):
                        ex.pop(0)()
                if spi == 0:
                    # mask qh1 column-halves ride the sync queue behind the
                    # x loads, one issue per early step
                    nc.sync.dma_start(mask_t[kc][:, HW:S],
                                      maskT[kc * P:(kc + 1) * P, HW:S])
                sc = scores_ps.tile([P, HW], f32, tag="sc")
                # K=128 via the duplicated operands: contraction sums the
                # head twice (q pre-scaled by 1/16 cancels it); full-array
                # activity keeps the HAM clock gate open
                for st in range(2):
                    nc.tensor.matmul(sc[:, ts(st, 512)],
                                     kd[h][:, ts(kc, P)],
                                     qd[h][:, q0 + st * 512: q0 + (st + 1) * 512],
                                     start=True, stop=True)
                pt = ppool.tile([P, HW], bf16, tag="p")
                nc.scalar.activation(pt[:], sc[:], AF.Exp)
                nc.vector.tensor_mul(pt[:], pt[:], mask_t[kc][:, q0:q0 + HW])
                if debug and i == 0:
                    nc.sync.dma_start(dbg_pt0[:], pt[:])
                if debug and i == 16:
                    nc.sync.dma_start(dbg_pt16[:], pt[:])
                pts[i] = pt

            def pv_stream(j):
                spi, h, qh, kc = steps[j]
                bv0 = (DK + 1) * h
                pt = pts.pop(j)
                for q2 in range(2):
                    nc.tensor.matmul(
                        xaugs[spi][:, ts(q2, 512)],
                        v_sb[:, kc * VG + bv0: kc * VG + bv0 + DK + 1],
                        pt[:, ts(q2, 512)],
                        start=(kc == 0), stop=(kc == KC - 1))
                if kc == KC - 1:
                    # inline evictions free the xaug banks before the next
                    # sub-phase's PV stream touches them: ScalarE ACT-copies
                    # the block, DVE computes 1/denominator from the psum
                    # row; bcast+mult are pushed into the NEXT sub-phase
                    xt = xtpool.tile([DK + 1, HW], f32, tag="xt")
                    nc.scalar.copy(xt[:], xaugs[spi][:])
                    if debug and spi == 0:
                        nc.sync.dma_start(dbg_xt0[:], xt[:])
                    if spi < 3:
                        extras[spi + 1].extend(norm_units(h, qh, xt))
                        if spi == 1:
                            extras[2].extend(oproj_unit(qb) for qb in range(8))
                    else:
                        tail_norm.extend(norm_units(h, qh, xt))

            def pv_allowed(j, i):
                if j > i - SKEW:
                    return False
                if j < KC:
                    # SP0: vproj(kc) is unit 2+kc, popped at SP0_POP0+(2+kc)//2
                    return i >= SP0_POP0 + (2 + j) // 2 + 1
                return True

            pv_next = 0
            i = 0
            while pv_next < len(steps):
                if i < len(steps):
                    sc_stream(i)
                for _ in range(2):  # catch-up: up to two PV steps per loop
                    if pv_next < len(steps) and pv_allowed(pv_next, i):
                        pv_stream(pv_next)
                        pv_next += 1
                i += 1

            # tail: last norm, then the second o-proj batch
            for u in tail_norm:
                u()
            for qb in range(8, 16):
                oproj_unit(qb, tail=(qb % 2 == 0))()

            if debug:
                nc.sync.dma_start(dbg_qd[:], qd[0][:])
                nc.sync.dma_start(dbg_kd[:], kd[0][:])
                nc.sync.dma_start(dbg_xhat[:], xhat[:])
                nc.sync.dma_start(dbg_vsb[:], v_sb[:])

    nc.compile()
    return nc


def _get_nc(S):
    if S not in _NC_CACHE:
        _NC_CACHE[S] = _build_nc(S)
    return _NC_CACHE[S]


def kernel(query, key, value, mask, Wq, bq, Wk, bk, Wv, bv, Wo, bo):
    global LAST_RESULTS
    trace = os.environ.get("MHA_TRACE", "0") == "1"
    if trace:
        _register_ntff_hook()

    from concourse.bass_utils import run_bass_kernel_spmd

    query = np.asarray(query)
    key = np.asarray(key)
    value = np.asarray(value)
    mask = np.asarray(mask)
    Wq, bq, Wk, bk = map(np.asarray, (Wq, bq, Wk, bk))
    Wv, bv, Wo, bo = map(np.asarray, (Wv, bv, Wo, bo))

    S = query.shape[1]
    nc = _get_nc(S)

    bf = ml_dtypes.bfloat16
    maskTb = np.ascontiguousarray((mask[0] != 0).T).astype(bf)
    xT = {}
    for b in range(B):
        xT[("q", b)] = np.ascontiguousarray(query[b].T).astype(bf)
        xT[("k", b)] = np.ascontiguousarray(key[b].T).astype(bf)
        xT[("v", b)] = np.ascontiguousarray(value[b].T).astype(bf)

    def w_chunks(Wsl):
        # [128 out, 512 e] -> [128 p(e%128), 512 (ec*128 + out)]
        return np.ascontiguousarray(
            Wsl.T.reshape(4, P, P).transpose(1, 0, 2).reshape(P, D))

    in_maps = []
    for c in range(N_CORES):
        b, hp = divmod(c, 4)
        sl = slice(P * hp, P * hp + P)
        wq_c = w_chunks(Wq[sl, :])
        wk_c = w_chunks(Wk[sl, :])
        wv_c = w_chunks(Wv[sl, :])
        wo_c = np.ascontiguousarray(Wo[:, sl].T)
        wcat = np.concatenate([wq_c, wk_c, wv_c, wo_c], axis=1).astype(bf)
        bqk = np.stack([bq[sl] / 16.0, bk[sl]], axis=1).astype(np.float32)
        in_maps.append({
            "xqT": xT[("q", b)],
            "xkT": xT[("k", b)],
            "xvT": xT[("v", b)],
            "maskT": maskTb,
            "wcat": wcat,
            "bqk": bqk,
            "bv_row": bv[sl].reshape(1, P).astype(bf),
        })

    res = run_bass_kernel_spmd(
        nc, in_maps, core_ids=list(range(N_CORES)),
        trace=trace, trace_cores=[0] if trace else None,
    )
    LAST_RESULTS = res

    out = np.zeros((B, S, D), np.float32)
    for c in range(N_CORES):
        out[c // 4] += res.results[c]["outp"].astype(np.float32)
    out += bo.astype(np.float32)
    return out


# revision 37
# speedup vs baseline: 1.3567x; 1.0278x over previous
"""Trainium2 Bass kernel for 8-head MHA (B=2, S=2048, d_model=512).

Sharding: core c -> batch b = c//4, head-pair hp = c%4 (heads 2hp, 2hp+1).
Each core computes q/k/v projections for its 128 out-dims (2 heads), the
masked-softmax attention for those heads, and the output-projection partial
for its 128 in-dims (heads summed on device). Host sums the 4 partials per
batch and adds the output bias.

Score matmuls use the duplicated-head K=128 form (head slice in both
partition halves of qd/kd, q pre-scaled by 1/16): full-array activity is
required to hold the PE HAM clock gate open -- K=64 row-tiled scores
measured ~1.8x slower per matmul because the array throttles to 1.2 GHz.

Head phase: only the first sub-phase's operands (fused wq|wk|wv|wo +
biases + xk half-0 on the scalar HWDGE queue, xq half-0 on sync) are
issued immediately; xk half-1, xv, xq half-1 and the qh0 mask tiles ride
the gpsimd SWDGE queue gated behind the xk-h0 load, because the 16 DMA
engines progress all queued transfers concurrently and ungated bulk
loads starve the critical path. ~24 dummy matmuls bridge the load phase
to keep the HAM warm. k/q half-1 projections run from the extras stream
through the misc psum pool with DVE tensor_scalar evictions so the
ScalarE exp stream is never blocked; the PV stream uses a catch-up
schedule gated on when each vproj unit was emitted so no matmul sits in
the static TensorE queue waiting on late xv/xq-h1 DMAs. PV eviction is
an inline ScalarE ACT-copy to an f32 xt tile (frees the single xaug psum
buffer before the next sub-phase's PV writes); normalization (K=1
ones-matmul broadcast + in-place DVE reciprocal + multiply) is deferred
one sub-phase as in the baseline. Mask qh1 column-halves and output
stores ride the sync queue during attention.
"""

import os
import sys
import types
import numpy as np
import ml_dtypes

HEAD = 8
D = 512
DK = 64
B = 2
N_CORES = 8
P = 128

_NC_CACHE = {}
LAST_RESULTS = None  # test harness reads BassKernelResults from here


def _register_ntff_hook():
    """Make run_bass_kernel_spmd(trace=True) work under axon by registering
    the NTFF profile hook that the trimmed antenv package lacks."""
    if "antenv.axon_hooks" in sys.modules:
        return
    try:
        import antenv

        mod = types.ModuleType("antenv.axon_hooks")
        _hook = [None]
        mod.set_axon_ntff_profile_hook = lambda h: _hook.__setitem__(0, h)
        mod.get_axon_ntff_profile_hook = lambda: _hook[0]
        sys.modules["antenv.axon_hooks"] = mod
        antenv.axon_hooks = mod
        if "/root/.axon_site" not in sys.path:
            sys.path.insert(0, "/root/.axon_site")
        from trn_agent_boot.trn_boot import _ntff_profile_via_ctypes

        mod.set_axon_ntff_profile_hook(
            _ntff_profile_via_ctypes("/opt/axon/libaxon_pjrt.so")
        )
    except Exception:
        pass  # tracing degrades; execution still works


def _build_nc(S):
    import concourse.tile as tile
    import concourse.mybir as mybir
    from concourse import bacc
    from concourse.bass import ts
    from contextlib import ExitStack

    from concourse.alu_op_type import AluOpType as Alu

    f32 = mybir.dt.float32
    bf16 = mybir.dt.bfloat16
    AF = mybir.ActivationFunctionType

    KC = S // P       # k chunks (score-tile rows == v s-blocks)
    EC = D // P       # embed chunks for projections
    HW = S // 2       # q half width (1024)
    VG = 130          # v_sb column group: [v_h0(64) | 1 | v_h1(64) | 1]
    NWARM = 40
    debug = os.environ.get("MHA_DEBUG", "0") == "1"

    nc = bacc.Bacc("TRN2", target_bir_lowering=False, debug=False,
                   num_devices=N_CORES)

    xqT = nc.dram_tensor("xqT", [D, S], bf16, kind="ExternalInput").ap()
    xkT = nc.dram_tensor("xkT", [D, S], bf16, kind="ExternalInput").ap()
    xvT = nc.dram_tensor("xvT", [D, S], bf16, kind="ExternalInput").ap()
    maskT = nc.dram_tensor("maskT", [S, S], bf16, kind="ExternalInput").ap()
    wcat = nc.dram_tensor("wcat", [P, 6 * D], bf16, kind="ExternalInput").ap()
    bqk = nc.dram_tensor("bqk", [P, 4], f32, kind="ExternalInput").ap()
    bv_row = nc.dram_tensor("bv_row", [1, P], bf16, kind="ExternalInput").ap()
    outp = nc.dram_tensor("outp", [S, D], bf16, kind="ExternalOutput").ap()
    if debug:
        dbg_qd = nc.dram_tensor("dbg_qd", [P, S], bf16, kind="ExternalOutput").ap()
        dbg_kd = nc.dram_tensor("dbg_kd", [P, S], bf16, kind="ExternalOutput").ap()
        dbg_pt0 = nc.dram_tensor("dbg_pt0", [P, S // 2], bf16, kind="ExternalOutput").ap()
        dbg_pt16 = nc.dram_tensor("dbg_pt16", [P, S // 2], bf16, kind="ExternalOutput").ap()
        dbg_xt0 = nc.dram_tensor("dbg_xt0", [DK + 1, S // 2], f32, kind="ExternalOutput").ap()
        dbg_xhat = nc.dram_tensor("dbg_xhat", [P, S], bf16, kind="ExternalOutput").ap()
        dbg_vsb = nc.dram_tensor("dbg_vsb", [P, (S // P) * VG], bf16, kind="ExternalOutput").ap()

    with tile.TileContext(nc) as tc, ExitStack() as ctx:
        consts = ctx.enter_context(tc.tile_pool(name="consts", bufs=1))
        resid = ctx.enter_context(tc.tile_pool(name="resid", bufs=1))
        mpool = ctx.enter_context(tc.tile_pool(name="maskp", bufs=KC))
        ppool = ctx.enter_context(tc.tile_pool(name="pp", bufs=10))
        xtpool = ctx.enter_context(tc.tile_pool(name="xtp", bufs=2))
        opool = ctx.enter_context(tc.tile_pool(name="outsb", bufs=4))

        ones_row = consts.tile([1, P], bf16)
        nc.vector.memset(ones_row[:], 1.0)
        ones1f = consts.tile([P, DK], f32)  # row DK used as bcast stationary
        nc.vector.memset(ones1f[:], 1.0)
        # [wqA | wqB | wkA | wkB | wv | wo]: q/k stationaries are per-head
        # duplicated on the host, so projections evict straight into the dup
        # layout with one ACT each and no SBUF->SBUF mirror DMAs
        wsb = consts.tile([P, 6 * D], bf16)
        bqk_sb = consts.tile([P, 4], f32)     # bqA/16 | bqB/16 | bkA | bkB
        bvr_sb = consts.tile([1, P], bf16)
        warm_sb = consts.tile([P, D], bf16)   # warmup matmul operands
        nc.vector.memset(warm_sb[:], 1.0)

        # residents: per-head duplicated q/k (head slice in BOTH partition
        # halves -> full-array K=128 score matmuls keep the PE HAM warm)
        qd = [resid.tile([P, S], bf16, name=f"qd{h}") for h in range(2)]
        kd = [resid.tile([P, S], bf16, name=f"kd{h}") for h in range(2)]
        v_sb = resid.tile([P, KC * VG], bf16)
        nc.vector.memset(v_sb[:], 1.0)  # pre-set the ones columns
        xhat = resid.tile([P, S], bf16)

        mask_t = [mpool.tile([P, S], bf16, tag="mask", name=f"mask{kc}")
                  for kc in range(KC)]

        # x tiles: xq/xk split by column half for early first-sub-phase start
        xqp = ctx.enter_context(tc.tile_pool(name="xqp", bufs=2 * EC))
        xkp = ctx.enter_context(tc.tile_pool(name="xkp", bufs=2 * EC))
        xvp = ctx.enter_context(tc.tile_pool(name="xvp", bufs=EC))
        xq_t = {}
        xk_t = {}
        for half in range(2):
            for ec in range(EC):
                xq_t[(half, ec)] = xqp.tile([P, HW], bf16, tag="xq", name=f"xq{half}_{ec}")
                xk_t[(half, ec)] = xkp.tile([P, HW], bf16, tag="xk", name=f"xk{half}_{ec}")
        xv_t = [xvp.tile([P, S], bf16, tag="xv", name=f"xv{ec}") for ec in range(EC)]

        # ---- projection section ----
        with tc.tile_pool(name="qk_ps", bufs=2, space="PSUM") as qk_ps, \
             tc.tile_pool(name="warm_ps", bufs=1, space="PSUM") as warm_ps:

            # DMA issue: weights on the scalar queue (tiny, first);
            # xq halves + xv on sync; xk halves on vector; mask qh0 via
            # SWDGE so 16 DMA engines ramp immediately on all fronts
            nc.scalar.dma_start(bqk_sb[:], bqk[:])
            nc.scalar.dma_start(bvr_sb[:], bv_row[:])
            nc.scalar.dma_start(wsb[:], wcat[:])
            for ec in range(EC):
                nc.sync.dma_start(xq_t[(0, ec)][:],
                                  xqT[ec * P:(ec + 1) * P, 0:HW])
            for ec in range(EC):
                nc.scalar.dma_start(xk_t[(0, ec)][:],
                                    xkT[ec * P:(ec + 1) * P, 0:HW])
            # everything that is not needed for the first sub-phase rides
            # SWDGE gated behind the critical xq/xk half-loads, so those get
            # the full HBM bandwidth (the 16 DMA engines otherwise progress
            # all queued transfers concurrently)
            mgate = consts.tile([1, 16], bf16)
            nc.gpsimd.tensor_copy(mgate[:], xk_t[(0, EC - 1)][0:1, 0:16])
            for ec in range(EC):
                nc.gpsimd.dma_start(xk_t[(1, ec)][:],
                                    xkT[ec * P:(ec + 1) * P, HW:S])
            for ec in range(EC):
                nc.gpsimd.dma_start(xq_t[(1, ec)][:],
                                    xqT[ec * P:(ec + 1) * P, HW:S])
            for ec in range(EC):
                nc.gpsimd.dma_start(xv_t[ec][:], xvT[ec * P:(ec + 1) * P, :])
            for kc in range(KC):
                nc.gpsimd.dma_start(mask_t[kc][:, 0:HW],
                                    maskT[kc * P:(kc + 1) * P, 0:HW])

            # PE warmup: keep the HAM clock gate open through the load phase
            wp = warm_ps.tile([P, D], f32)
            for i in range(NWARM):
                nc.tensor.matmul(wp[:, 0:D], warm_sb[:, 0:P],
                                 warm_sb[:, 0:D], start=True, stop=True)

            def proj_matmuls(w0, x_half, half):
                ps = qk_ps.tile([P, HW], f32, tag="qk")
                for ec in range(EC):
                    for st in range(2):
                        nc.tensor.matmul(
                            ps[:, ts(st, 512)], wsb[:, w0 + ec * P: w0 + (ec + 1) * P],
                            x_half[(half, ec)][:, ts(st, 512)],
                            start=(ec == 0), stop=(ec == EC - 1))
                return ps

            # half-0 passes evict on ScalarE (before any exp); pass A
            # (head 0) of q and k go first -- the first sub-phase needs only
            # qd[0]/kd[0], so attention starts as early as possible
            def proj_pass(w0, x_half, half, dst, bias_col, scale):
                ps = proj_matmuls(w0, x_half, half)
                nc.scalar.activation(dst[:, half * HW:(half + 1) * HW], ps[:],
                                     AF.Identity,
                                     bias=bqk_sb[:, bias_col:bias_col + 1],
                                     scale=scale)

            proj_pass(0 * D, xq_t, 0, qd[0], 0, 0.0625)
            proj_pass(2 * D, xk_t, 0, kd[0], 2, 1.0)
            proj_pass(1 * D, xq_t, 0, qd[1], 1, 0.0625)
            proj_pass(3 * D, xk_t, 0, kd[1], 3, 1.0)

        # ---- attention: flat two-stream pipeline, SKEW-step lag ----
        # (qk_ps/warm_ps are closed: PSUM budget = sc 4 + xaug 2 + misc 2)
        scores_ps = ctx.enter_context(
            tc.tile_pool(name="sc_ps", bufs=2, space="PSUM"))
        xaug_ps = ctx.enter_context(
            tc.tile_pool(name="xa_ps", bufs=1, space="PSUM"))
        misc_ps = ctx.enter_context(
            tc.tile_pool(name="mi_ps", bufs=2, space="PSUM"))

        if True:
            # k/q half1 land mid-attention: run them from the extras stream
            # in 512-col pieces through the misc pool, evicting on DVE so
            # the ScalarE exp stream is never blocked
            def proj_late_unit(w0, dstt, st, scale, bias_col):
                x_half = xk_t if w0 >= 2 * D else xq_t

                def emit():
                    ps = misc_ps.tile([P, 512], f32, tag="mi")
                    for ec in range(EC):
                        nc.tensor.matmul(
                            ps[:], wsb[:, w0 + ec * P: w0 + (ec + 1) * P],
                            x_half[(1, ec)][:, ts(st, 512)],
                            start=(ec == 0), stop=(ec == EC - 1))
                    c0 = HW + st * 512
                    if scale is None:
                        nc.vector.tensor_scalar(
                            out=dstt[:, c0:c0 + 512], in0=ps[:],
                            scalar1=bqk_sb[:, bias_col:bias_col + 1],
                            scalar2=None, op0=Alu.add)
                    else:
                        nc.vector.tensor_scalar(
                            out=dstt[:, c0:c0 + 512], in0=ps[:], scalar1=scale,
                            scalar2=bqk_sb[:, bias_col:bias_col + 1],
                            op0=Alu.mult, op1=Alu.add)
                return emit

            # order: kd[0] pieces first (SP0 scores kc8-15 need them soonest)
            late_proj = [
                proj_late_unit(2 * D, kd[0], 0, None, 2),
                proj_late_unit(2 * D, kd[0], 1, None, 2),
                proj_late_unit(3 * D, kd[1], 0, None, 3),
                proj_late_unit(3 * D, kd[1], 1, None, 3),
                proj_late_unit(0 * D, qd[0], 0, 0.0625, 0),
                proj_late_unit(0 * D, qd[0], 1, 0.0625, 0),
                proj_late_unit(1 * D, qd[1], 0, 0.0625, 1),
                proj_late_unit(1 * D, qd[1], 1, 0.0625, 1),
            ]

            def vproj_unit(sb):
                def emit():
                    vpt = misc_ps.tile([P, 512], f32, tag="mi")
                    for ec in range(EC):
                        nc.tensor.matmul(vpt[:, 0:P],
                                         xv_t[ec][:, ts(sb, P)],
                                         wsb[:, 4 * D + ec * P: 4 * D + (ec + 1) * P],
                                         start=(ec == 0), stop=False)
                    nc.tensor.matmul(vpt[:, 0:P], ones_row[:], bvr_sb[:],
                                     start=False, stop=True)
                    nc.vector.tensor_copy(v_sb[:, sb * VG: sb * VG + DK],
                                          vpt[:, 0:DK])
                    nc.vector.tensor_copy(
                        v_sb[:, sb * VG + DK + 1: sb * VG + 2 * DK + 1],
                        vpt[:, DK:2 * DK])
                return emit

            def oproj_unit(qb, tail=False):
                def emit():
                    op = misc_ps.tile([P, 512], f32, tag="mi")
                    nc.tensor.matmul(op[:], xhat[:, ts(qb, P)],
                                     wsb[:, 5 * D:6 * D], start=True, stop=True)
                    ob = opool.tile([P, D], bf16, tag="ob")
                    if tail:  # ScalarE is idle after the last exp
                        nc.scalar.copy(ob[:], op[:])
                    else:
                        nc.vector.tensor_copy(ob[:], op[:])
                    nc.sync.dma_start(outp[qb * P:(qb + 1) * P, :], ob[:])
                return emit

            def norm_units(h, qh, xt):
                # denominator row broadcast via K=1 matmul into misc psum,
                # reciprocal in place (partition-base-0, HW-proven), then
                # normalize into xhat -- two 512-col units per sub-phase
                q0 = qh * HW
                units = []
                for q2 in range(2):
                    def emit(q2=q2):
                        bc = misc_ps.tile([DK, 512], f32, tag="mi")
                        nc.tensor.matmul(bc[:], ones1f[DK:DK + 1, :],
                                         xt[DK:DK + 1, ts(q2, 512)],
                                         start=True, stop=True)
                        nc.vector.reciprocal_approx_fast(out=bc[:], in_=bc[:])
                        nc.vector.tensor_mul(
                            xhat[h * DK:(h + 1) * DK,
                                 q0 + q2 * 512: q0 + (q2 + 1) * 512],
                            xt[0:DK, ts(q2, 512)], bc[:])
                    units.append(emit)
                return units

            SPs = [(0, 0), (1, 0), (0, 1), (1, 1)]
            steps = [(spi, h, qh, kc)
                     for spi, (h, qh) in enumerate(SPs) for kc in range(KC)]
            SKEW = 3
            # SP0 extras pop 2/step starting at step 5, so no unit's matmuls
            # sit in the static TensorE queue waiting on late xv/xq-h1 DMAs
            SP0_POP0 = 4
            extras = {0: late_proj + [vproj_unit(sb) for sb in range(KC)],
                      1: [], 2: [], 3: []}
            pts = {}
            xaugs = {}
            tail_norm = []

            def sc_stream(i):
                spi, h, qh, kc = steps[i]
                q0 = qh * HW
                if kc == 0:
                    xaugs[spi] = xaug_ps.tile([DK + 1, HW], f32, tag="xaug", name=f"xa{spi}")
                ex = extras[spi]
                for _ in range(2 if spi == 0 else 1):
                    if ex and (kc >= SP0_POP0 if spi == 0 else kc >= 2):
                        ex.pop(0)()
                if spi == 0:
                    # mask qh1 column-halves ride the sync queue behind the
                    # x loads, one issue per early step
                    nc.sync.dma_start(mask_t[kc][:, HW:S],
                                      maskT[kc * P:(kc + 1) * P, HW:S])
                sc = scores_ps.tile([P, HW], f32, tag="sc")
                # K=128 via the duplicated operands: contraction sums the
                # head twice (q pre-scaled by 1/16 cancels it); full-array
                # activity keeps the HAM clock gate open
                for st in range(2):
                    nc.tensor.matmul(sc[:, ts(st, 512)],
                                     kd[h][:, ts(kc, P)],
                                     qd[h][:, q0 + st * 512: q0 + (st + 1) * 512],
                                     start=True, stop=True)
                pt = ppool.tile([P, HW], bf16, tag="p")
                nc.scalar.activation(pt[:], sc[:], AF.Exp)
                nc.vector.tensor_mul(pt[:], pt[:], mask_t[kc][:, q0:q0 + HW])
                if debug and i == 0:
                    nc.sync.dma_start(dbg_pt0[:], pt[:])
                if debug and i == 16:
                    nc.sync.dma_start(dbg_pt16[:], pt[:])
                pts[i] = pt

            def pv_stream(j):
                spi, h, qh, kc = steps[j]
                bv0 = (DK + 1) * h
                pt = pts.pop(j)
                for q2 in range(2):
                    nc.tensor.matmul(
                        xaugs[spi][:, ts(q2, 512)],
                        v_sb[:, kc * VG + bv0: kc * VG + bv0 + DK + 1],
                        pt[:, ts(q2, 512)],
                        start=(kc == 0), stop=(kc == KC - 1))
                if kc == KC - 1:
                    # inline evictions free the xaug banks before the next
                    # sub-phase's PV stream touches them: ScalarE ACT-copies
                    # the block, DVE computes 1/denominator from the psum
                    # row; bcast+mult are pushed into the NEXT sub-phase
                    xt = xtpool.tile([DK + 1, HW], f32, tag="xt")
                    nc.scalar.copy(xt[:], xaugs[spi][:])
                    if debug and spi == 0:
                        nc.sync.dma_start(dbg_xt0[:], xt[:])
                    if spi < 3:
                        extras[spi + 1].extend(norm_units(h, qh, xt))
                        if spi == 1:
                            extras[2].extend(oproj_unit(qb) for qb in range(8))
                    else:
                        tail_norm.extend(norm_units(h, qh, xt))

            def pv_allowed(j, i):
                if j > i - SKEW:
                    return False
                if j < KC:
                    # SP0: vproj(kc) is unit 8+kc, popped at SP0_POP0+(8+kc)//2
                    return i >= SP0_POP0 + (8 + j) // 2 + 1
                return True

            pv_next = 0
            i = 0
            while pv_next < len(steps):
                if i < len(steps):
                    sc_stream(i)
                for _ in range(2):  # catch-up: up to two PV steps per loop
                    if pv_next < len(steps) and pv_allowed(pv_next, i):
                        pv_stream(pv_next)
                        pv_next += 1
                i += 1

            # tail: last norm, then the second o-proj batch
            for u in tail_norm:
                u()
            for qb in range(8, 16):
                oproj_unit(qb, tail=(qb % 2 == 0))()

            if debug:
                nc.sync.dma_start(dbg_qd[:], qd[0][:])
                nc.sync.dma_start(dbg_kd[:], kd[0][:])
                nc.sync.dma_start(dbg_xhat[:], xhat[:])
                nc.sync.dma_start(dbg_vsb[:], v_sb[:])

    nc.compile()
    return nc


def _get_nc(S):
    if S not in _NC_CACHE:
        _NC_CACHE[S] = _build_nc(S)
    return _NC_CACHE[S]


def kernel(query, key, value, mask, Wq, bq, Wk, bk, Wv, bv, Wo, bo):
    global LAST_RESULTS
    trace = os.environ.get("MHA_TRACE", "0") == "1"
    if trace:
        _register_ntff_hook()

    from concourse.bass_utils import run_bass_kernel_spmd

    query = np.asarray(query)
    key = np.asarray(key)
    value = np.asarray(value)
    mask = np.asarray(mask)
    Wq, bq, Wk, bk = map(np.asarray, (Wq, bq, Wk, bk))
    Wv, bv, Wo, bo = map(np.asarray, (Wv, bv, Wo, bo))

    S = query.shape[1]
    nc = _get_nc(S)

    bf = ml_dtypes.bfloat16
    maskTb = np.ascontiguousarray((mask[0] != 0).T).astype(bf)
    xT = {}
    for b in range(B):
        xT[("q", b)] = np.ascontiguousarray(query[b].T).astype(bf)
        xT[("k", b)] = np.ascontiguousarray(key[b].T).astype(bf)
        xT[("v", b)] = np.ascontiguousarray(value[b].T).astype(bf)

    def w_chunks(Wsl):
        # [128 out, 512 e] -> [128 p(e%128), 512 (ec*128 + out)]
        return np.ascontiguousarray(
            Wsl.T.reshape(4, P, P).transpose(1, 0, 2).reshape(P, D))

    in_maps = []
    for c in range(N_CORES):
        b, hp = divmod(c, 4)
        sl = slice(P * hp, P * hp + P)
        def dup(Wsl, h):
            return w_chunks(np.concatenate([Wsl[h * 64:(h + 1) * 64]] * 2, 0))

        wv_c = w_chunks(Wv[sl, :])
        wo_c = np.ascontiguousarray(Wo[:, sl].T)
        wcat = np.concatenate(
            [dup(Wq[sl], 0), dup(Wq[sl], 1), dup(Wk[sl], 0), dup(Wk[sl], 1),
             wv_c, wo_c], axis=1).astype(bf)

        def dupb(b, h):
            return np.concatenate([b[h * 64:(h + 1) * 64]] * 2, 0)

        bqk = np.stack([dupb(bq[sl], 0) / 16.0, dupb(bq[sl], 1) / 16.0,
                        dupb(bk[sl], 0), dupb(bk[sl], 1)],
                       axis=1).astype(np.float32)
        in_maps.append({
            "xqT": xT[("q", b)],
            "xkT": xT[("k", b)],
            "xvT": xT[("v", b)],
            "maskT": maskTb,
            "wcat": wcat,
            "bqk": bqk,
            "bv_row": bv[sl].reshape(1, P).astype(bf),
        })

    res = run_bass_kernel_spmd(
        nc, in_maps, core_ids=list(range(N_CORES)),
        trace=trace, trace_cores=[0] if trace else None,
    )
    LAST_RESULTS = res

    out = np.zeros((B, S, D), np.float32)
    for c in range(N_CORES):
        out[c // 4] += res.results[c]["outp"].astype(np.float32)
    out += bo.astype(np.float32)
    return out


# revision 38
# speedup vs baseline: 1.3624x; 1.0042x over previous
"""Trainium2 Bass kernel for 8-head MHA (B=2, S=2048, d_model=512).

Sharding: core c -> batch b = c//4, head-pair hp = c%4 (heads 2hp, 2hp+1).
Each core computes q/k/v projections for its 128 out-dims (2 heads), the
masked-softmax attention for those heads, and the output-projection partial
for its 128 in-dims (heads summed on device). Host sums the 4 partials per
batch and adds the output bias.

Score matmuls use the duplicated-head K=128 form (head slice in both
partition halves of qd/kd, q pre-scaled by 1/16): full-array activity is
required to hold the PE HAM clock gate open -- K=64 row-tiled scores
measured ~1.8x slower per matmul because the array throttles to 1.2 GHz.

Head phase: only the first sub-phase's operands (fused wq|wk|wv|wo +
biases + xk half-0 on the scalar HWDGE queue, xq half-0 on sync) are
issued immediately; xk half-1, xv, xq half-1 and the qh0 mask tiles ride
the gpsimd SWDGE queue gated behind the xk-h0 load, because the 16 DMA
engines progress all queued transfers concurrently and ungated bulk
loads starve the critical path. ~24 dummy matmuls bridge the load phase
to keep the HAM warm. k/q half-1 projections run from the extras stream
through the misc psum pool with DVE tensor_scalar evictions so the
ScalarE exp stream is never blocked; the PV stream uses a catch-up
schedule gated on when each vproj unit was emitted so no matmul sits in
the static TensorE queue waiting on late xv/xq-h1 DMAs. PV eviction is
an inline ScalarE ACT-copy to an f32 xt tile (frees the single xaug psum
buffer before the next sub-phase's PV writes); normalization (K=1
ones-matmul broadcast + in-place DVE reciprocal + multiply) is deferred
one sub-phase as in the baseline. Mask qh1 column-halves and output
stores ride the sync queue during attention.
"""

import os
import sys
import types
import numpy as np
import ml_dtypes

HEAD = 8
D = 512
DK = 64
B = 2
N_CORES = 8
P = 128

_NC_CACHE = {}
LAST_RESULTS = None  # test harness reads BassKernelResults from here


def _register_ntff_hook():
    """Make run_bass_kernel_spmd(trace=True) work under axon by registering
    the NTFF profile hook that the trimmed antenv package lacks."""
    if "antenv.axon_hooks" in sys.modules:
        return
    try:
        import antenv

        mod = types.ModuleType("antenv.axon_hooks")
        _hook = [None]
        mod.set_axon_ntff_profile_hook = lambda h: _hook.__setitem__(0, h)
        mod.get_axon_ntff_profile_hook = lambda: _hook[0]
        sys.modules["antenv.axon_hooks"] = mod
        antenv.axon_hooks = mod
        if "/root/.axon_site" not in sys.path:
            sys.path.insert(0, "/root/.axon_site")
        from trn_agent_boot.trn_boot import _ntff_profile_via_ctypes

        mod.set_axon_ntff_profile_hook(
            _ntff_profile_via_ctypes("/opt/axon/libaxon_pjrt.so")
        )
    except Exception:
        pass  # tracing degrades; execution still works


def _build_nc(S):
    import concourse.tile as tile
    import concourse.mybir as mybir
    from concourse import bacc
    from concourse.bass import ts
    from contextlib import ExitStack

    from concourse.alu_op_type import AluOpType as Alu

    f32 = mybir.dt.float32
    bf16 = mybir.dt.bfloat16
    AF = mybir.ActivationFunctionType

    KC = S // P       # k chunks (score-tile rows == v s-blocks)
    EC = D // P       # embed chunks for projections
    HW = S // 2       # q half width (1024)
    VG = 130          # v_sb column group: [v_h0(64) | 1 | v_h1(64) | 1]
    NWARM = 40
    debug = os.environ.get("MHA_DEBUG", "0") == "1"

    nc = bacc.Bacc("TRN2", target_bir_lowering=False, debug=False,
                   num_devices=N_CORES)

    xqT = nc.dram_tensor("xqT", [D, S], bf16, kind="ExternalInput").ap()
    xkT = nc.dram_tensor("xkT", [D, S], bf16, kind="ExternalInput").ap()
    xvT = nc.dram_tensor("xvT", [D, S], bf16, kind="ExternalInput").ap()
    maskT = nc.dram_tensor("maskT", [S, S], bf16, kind="ExternalInput").ap()
    wcat = nc.dram_tensor("wcat", [P, 6 * D], bf16, kind="ExternalInput").ap()
    bqk = nc.dram_tensor("bqk", [P, 4], f32, kind="ExternalInput").ap()
    bv_row = nc.dram_tensor("bv_row", [1, P], bf16, kind="ExternalInput").ap()
    outp = nc.dram_tensor("outp", [S, D], bf16, kind="ExternalOutput").ap()
    if debug:
        dbg_qd = nc.dram_tensor("dbg_qd", [P, S], bf16, kind="ExternalOutput").ap()
        dbg_kd = nc.dram_tensor("dbg_kd", [P, S], bf16, kind="ExternalOutput").ap()
        dbg_pt0 = nc.dram_tensor("dbg_pt0", [P, S // 2], bf16, kind="ExternalOutput").ap()
        dbg_pt16 = nc.dram_tensor("dbg_pt16", [P, S // 2], bf16, kind="ExternalOutput").ap()
        dbg_xt0 = nc.dram_tensor("dbg_xt0", [DK + 1, S // 2], f32, kind="ExternalOutput").ap()
        dbg_xhat = nc.dram_tensor("dbg_xhat", [P, S], bf16, kind="ExternalOutput").ap()
        dbg_vsb = nc.dram_tensor("dbg_vsb", [P, (S // P) * VG], bf16, kind="ExternalOutput").ap()

    with tile.TileContext(nc) as tc, ExitStack() as ctx:
        consts = ctx.enter_context(tc.tile_pool(name="consts", bufs=1))
        resid = ctx.enter_context(tc.tile_pool(name="resid", bufs=1))
        mpool = ctx.enter_context(tc.tile_pool(name="maskp", bufs=KC))
        ppool = ctx.enter_context(tc.tile_pool(name="pp", bufs=16))
        xtpool = ctx.enter_context(tc.tile_pool(name="xtp", bufs=2))
        opool = ctx.enter_context(tc.tile_pool(name="outsb", bufs=4))

        ones_row = consts.tile([1, P], bf16)
        nc.vector.memset(ones_row[:], 1.0)
        ones1f = consts.tile([P, DK], f32)  # row DK used as bcast stationary
        nc.vector.memset(ones1f[:], 1.0)
        # [wqA | wqB | wkA | wkB | wv | wo]: q/k stationaries are per-head
        # duplicated on the host, so projections evict straight into the dup
        # layout with one ACT each and no SBUF->SBUF mirror DMAs
        wsb = consts.tile([P, 6 * D], bf16)
        bqk_sb = consts.tile([P, 4], f32)     # bqA/16 | bqB/16 | bkA | bkB
        bvr_sb = consts.tile([1, P], bf16)
        warm_sb = consts.tile([P, D], bf16)   # warmup matmul operands
        nc.vector.memset(warm_sb[:], 1.0)

        # residents: per-head duplicated q/k (head slice in BOTH partition
        # halves -> full-array K=128 score matmuls keep the PE HAM warm)
        qd = [resid.tile([P, S], bf16, name=f"qd{h}") for h in range(2)]
        kd = [resid.tile([P, S], bf16, name=f"kd{h}") for h in range(2)]
        v_sb = resid.tile([P, KC * VG], bf16)
        nc.vector.memset(v_sb[:], 1.0)  # pre-set the ones columns
        xhat = resid.tile([P, S], bf16)

        mask_t = [mpool.tile([P, S], bf16, tag="mask", name=f"mask{kc}")
                  for kc in range(KC)]

        # x tiles: xq/xk split by column half for early first-sub-phase start
        xqp = ctx.enter_context(tc.tile_pool(name="xqp", bufs=2 * EC))
        xkp = ctx.enter_context(tc.tile_pool(name="xkp", bufs=2 * EC))
        xvp = ctx.enter_context(tc.tile_pool(name="xvp", bufs=EC))
        xq_t = {}
        xk_t = {}
        for half in range(2):
            for ec in range(EC):
                xq_t[(half, ec)] = xqp.tile([P, HW], bf16, tag="xq", name=f"xq{half}_{ec}")
                xk_t[(half, ec)] = xkp.tile([P, HW], bf16, tag="xk", name=f"xk{half}_{ec}")
        xv_t = [xvp.tile([P, S], bf16, tag="xv", name=f"xv{ec}") for ec in range(EC)]

        # ---- projection section ----
        with tc.tile_pool(name="qk_ps", bufs=2, space="PSUM") as qk_ps, \
             tc.tile_pool(name="warm_ps", bufs=1, space="PSUM") as warm_ps:

            # DMA issue: weights on the scalar queue (tiny, first);
            # xq halves + xv on sync; xk halves on vector; mask qh0 via
            # SWDGE so 16 DMA engines ramp immediately on all fronts
            nc.scalar.dma_start(bqk_sb[:], bqk[:])
            nc.scalar.dma_start(bvr_sb[:], bv_row[:])
            nc.scalar.dma_start(wsb[:], wcat[:])
            for ec in range(EC):
                nc.sync.dma_start(xq_t[(0, ec)][:],
                                  xqT[ec * P:(ec + 1) * P, 0:HW])
            for ec in range(EC):
                nc.scalar.dma_start(xk_t[(0, ec)][:],
                                    xkT[ec * P:(ec + 1) * P, 0:HW])
            # everything that is not needed for the first sub-phase rides
            # SWDGE gated behind the critical xq/xk half-loads, issued in
            # deadline order (the 16 DMA engines otherwise progress all
            # queued transfers concurrently and starve the critical path)
            mgate = consts.tile([1, 16], bf16)
            nc.gpsimd.tensor_copy(mgate[:], xk_t[(0, EC - 1)][0:1, 0:16])
            for kc in range(8):
                nc.gpsimd.dma_start(mask_t[kc][:, 0:HW],
                                    maskT[kc * P:(kc + 1) * P, 0:HW])
            for ec in range(EC):
                nc.gpsimd.dma_start(xk_t[(1, ec)][:],
                                    xkT[ec * P:(ec + 1) * P, HW:S])
            for kc in range(8, KC):
                nc.gpsimd.dma_start(mask_t[kc][:, 0:HW],
                                    maskT[kc * P:(kc + 1) * P, 0:HW])
            for ec in range(EC):
                nc.gpsimd.dma_start(xq_t[(1, ec)][:],
                                    xqT[ec * P:(ec + 1) * P, HW:S])
            for ec in range(EC):
                nc.gpsimd.dma_start(xv_t[ec][:], xvT[ec * P:(ec + 1) * P, :])
            # mask qh1 halves go on the sync queue, but each is WAW-gated by
            # a tiny gpsimd write (dependent on the last SWDGE transfer) so
            # the sync sequencer cannot issue them before the head is done
            mh1gate = consts.tile([1, 16], bf16)
            nc.gpsimd.tensor_copy(mh1gate[:], xv_t[EC - 1][0:1, 0:16])
            for kc in range(KC):
                nc.gpsimd.tensor_copy(mask_t[kc][0:1, HW:HW + 16], mh1gate[:])

            # PE warmup: keep the HAM clock gate open through the load phase
            wp = warm_ps.tile([P, D], f32)
            for i in range(NWARM):
                nc.tensor.matmul(wp[:, 0:D], warm_sb[:, 0:P],
                                 warm_sb[:, 0:D], start=True, stop=True)

            def proj_matmuls(w0, x_half, half):
                ps = qk_ps.tile([P, HW], f32, tag="qk")
                for ec in range(EC):
                    for st in range(2):
                        nc.tensor.matmul(
                            ps[:, ts(st, 512)], wsb[:, w0 + ec * P: w0 + (ec + 1) * P],
                            x_half[(half, ec)][:, ts(st, 512)],
                            start=(ec == 0), stop=(ec == EC - 1))
                return ps

            # half-0 passes evict on ScalarE (before any exp); pass A
            # (head 0) of q and k go first -- the first sub-phase needs only
            # qd[0]/kd[0], so attention starts as early as possible
            def proj_pass(w0, x_half, half, dst, bias_col, scale):
                ps = proj_matmuls(w0, x_half, half)
                nc.scalar.activation(dst[:, half * HW:(half + 1) * HW], ps[:],
                                     AF.Identity,
                                     bias=bqk_sb[:, bias_col:bias_col + 1],
                                     scale=scale)

            proj_pass(0 * D, xq_t, 0, qd[0], 0, 0.0625)
            proj_pass(2 * D, xk_t, 0, kd[0], 2, 1.0)
            proj_pass(1 * D, xq_t, 0, qd[1], 1, 0.0625)
            proj_pass(3 * D, xk_t, 0, kd[1], 3, 1.0)

        # ---- attention: flat two-stream pipeline, SKEW-step lag ----
        # (qk_ps/warm_ps are closed: PSUM budget = sc 4 + xaug 2 + misc 2)
        scores_ps = ctx.enter_context(
            tc.tile_pool(name="sc_ps", bufs=2, space="PSUM"))
        xaug_ps = ctx.enter_context(
            tc.tile_pool(name="xa_ps", bufs=1, space="PSUM"))
        misc_ps = ctx.enter_context(
            tc.tile_pool(name="mi_ps", bufs=2, space="PSUM"))

        if True:
            # k/q half1 land mid-attention: run them from the extras stream
            # in 512-col pieces through the misc pool, evicting on DVE so
            # the ScalarE exp stream is never blocked
            def proj_late_unit(w0, dstt, st, scale, bias_col):
                x_half = xk_t if w0 >= 2 * D else xq_t

                def emit():
                    ps = misc_ps.tile([P, 512], f32, tag="mi")
                    for ec in range(EC):
                        nc.tensor.matmul(
                            ps[:], wsb[:, w0 + ec * P: w0 + (ec + 1) * P],
                            x_half[(1, ec)][:, ts(st, 512)],
                            start=(ec == 0), stop=(ec == EC - 1))
                    c0 = HW + st * 512
                    if scale is None:
                        nc.vector.tensor_scalar(
                            out=dstt[:, c0:c0 + 512], in0=ps[:],
                            scalar1=bqk_sb[:, bias_col:bias_col + 1],
                            scalar2=None, op0=Alu.add)
                    else:
                        nc.vector.tensor_scalar(
                            out=dstt[:, c0:c0 + 512], in0=ps[:], scalar1=scale,
                            scalar2=bqk_sb[:, bias_col:bias_col + 1],
                            op0=Alu.mult, op1=Alu.add)
                return emit

            # order: kd[0] pieces first (SP0 scores kc8-15 need them soonest)
            late_proj = [
                proj_late_unit(2 * D, kd[0], 0, None, 2),
                proj_late_unit(2 * D, kd[0], 1, None, 2),
                proj_late_unit(3 * D, kd[1], 0, None, 3),
                proj_late_unit(3 * D, kd[1], 1, None, 3),
                proj_late_unit(0 * D, qd[0], 0, 0.0625, 0),
                proj_late_unit(0 * D, qd[0], 1, 0.0625, 0),
                proj_late_unit(1 * D, qd[1], 0, 0.0625, 1),
                proj_late_unit(1 * D, qd[1], 1, 0.0625, 1),
            ]

            def vproj_unit(sb):
                def emit():
                    vpt = misc_ps.tile([P, 512], f32, tag="mi")
                    for ec in range(EC):
                        nc.tensor.matmul(vpt[:, 0:P],
                                         xv_t[ec][:, ts(sb, P)],
                                         wsb[:, 4 * D + ec * P: 4 * D + (ec + 1) * P],
                                         start=(ec == 0), stop=False)
                    nc.tensor.matmul(vpt[:, 0:P], ones_row[:], bvr_sb[:],
                                     start=False, stop=True)
                    nc.vector.tensor_copy(v_sb[:, sb * VG: sb * VG + DK],
                                          vpt[:, 0:DK])
                    nc.vector.tensor_copy(
                        v_sb[:, sb * VG + DK + 1: sb * VG + 2 * DK + 1],
                        vpt[:, DK:2 * DK])
                return emit

            def oproj_unit(qb, tail=False):
                def emit():
                    op = misc_ps.tile([P, 512], f32, tag="mi")
                    nc.tensor.matmul(op[:], xhat[:, ts(qb, P)],
                                     wsb[:, 5 * D:6 * D], start=True, stop=True)
                    ob = opool.tile([P, D], bf16, tag="ob")
                    if tail:  # ScalarE is idle after the last exp
                        nc.scalar.copy(ob[:], op[:])
                    else:
                        nc.vector.tensor_copy(ob[:], op[:])
                    nc.sync.dma_start(outp[qb * P:(qb + 1) * P, :], ob[:])
                return emit

            def norm_units(h, qh, xt):
                # denominator row broadcast via K=1 matmul into misc psum,
                # reciprocal in place (partition-base-0, HW-proven), then
                # normalize into xhat -- two 512-col units per sub-phase
                q0 = qh * HW
                units = []
                for q2 in range(2):
                    def emit(q2=q2):
                        bc = misc_ps.tile([DK, 512], f32, tag="mi")
                        nc.tensor.matmul(bc[:], ones1f[DK:DK + 1, :],
                                         xt[DK:DK + 1, ts(q2, 512)],
                                         start=True, stop=True)
                        nc.vector.reciprocal_approx_fast(out=bc[:], in_=bc[:])
                        nc.vector.tensor_mul(
                            xhat[h * DK:(h + 1) * DK,
                                 q0 + q2 * 512: q0 + (q2 + 1) * 512],
                            xt[0:DK, ts(q2, 512)], bc[:])
                    units.append(emit)
                return units

            SPs = [(0, 0), (1, 0), (0, 1), (1, 1)]
            steps = [(spi, h, qh, kc)
                     for spi, (h, qh) in enumerate(SPs) for kc in range(KC)]
            SKEW = 3
            # SP0 extras pop 2/step starting at step 5, so no unit's matmuls
            # sit in the static TensorE queue waiting on late xv/xq-h1 DMAs
            SP0_POP0 = 4
            vproj_popped = {}
            extras = {0: late_proj + [(sb, vproj_unit(sb)) for sb in range(KC)],
                      1: [], 2: [], 3: []}
            pts = {}
            xaugs = {}
            tail_norm = []

            def sc_stream(i):
                spi, h, qh, kc = steps[i]
                q0 = qh * HW
                if kc == 0:
                    xaugs[spi] = xaug_ps.tile([DK + 1, HW], f32, tag="xaug", name=f"xa{spi}")
                # leftover SP0 units (vproj waiting on late xv) spill into
                # later sub-phases' pop slots; record when each vproj lands
                for _ in range(2):
                    ex = extras[0] if extras[0] else extras[spi]
                    if ex and (kc >= SP0_POP0 if spi == 0 else kc >= 2):
                        u = ex.pop(0)
                        if isinstance(u, tuple):
                            vproj_popped[u[0]] = i
                            u[1]()
                        else:
                            u()
                if spi == 0:
                    # mask qh1 column-halves ride the sync queue behind the
                    # x loads, one issue per early step
                    nc.sync.dma_start(mask_t[kc][:, HW:S],
                                      maskT[kc * P:(kc + 1) * P, HW:S])
                sc = scores_ps.tile([P, HW], f32, tag="sc")
                # K=128 via the duplicated operands: contraction sums the
                # head twice (q pre-scaled by 1/16 cancels it); full-array
                # activity keeps the HAM clock gate open
                for st in range(2):
                    nc.tensor.matmul(sc[:, ts(st, 512)],
                                     kd[h][:, ts(kc, P)],
                                     qd[h][:, q0 + st * 512: q0 + (st + 1) * 512],
                                     start=True, stop=True)
                pt = ppool.tile([P, HW], bf16, tag="p")
                nc.scalar.activation(pt[:], sc[:], AF.Exp)
                nc.vector.tensor_mul(pt[:], pt[:], mask_t[kc][:, q0:q0 + HW])
                if debug and i == 0:
                    nc.sync.dma_start(dbg_pt0[:], pt[:])
                if debug and i == 16:
                    nc.sync.dma_start(dbg_pt16[:], pt[:])
                pts[i] = pt

            def pv_stream(j):
                spi, h, qh, kc = steps[j]
                bv0 = (DK + 1) * h
                pt = pts.pop(j)
                for q2 in range(2):
                    nc.tensor.matmul(
                        xaugs[spi][:, ts(q2, 512)],
                        v_sb[:, kc * VG + bv0: kc * VG + bv0 + DK + 1],
                        pt[:, ts(q2, 512)],
                        start=(kc == 0), stop=(kc == KC - 1))
                if kc == KC - 1:
                    # inline evictions free the xaug banks before the next
                    # sub-phase's PV stream touches them: ScalarE ACT-copies
                    # the block, DVE computes 1/denominator from the psum
                    # row; bcast+mult are pushed into the NEXT sub-phase
                    xt = xtpool.tile([DK + 1, HW], f32, tag="xt")
                    nc.scalar.copy(xt[:], xaugs[spi][:])
                    if debug and spi == 0:
                        nc.sync.dma_start(dbg_xt0[:], xt[:])
                    if spi < 3:
                        extras[spi + 1].extend(norm_units(h, qh, xt))
                        if spi == 1:
                            extras[2].extend(oproj_unit(qb) for qb in range(8))
                    else:
                        tail_norm.extend(norm_units(h, qh, xt))

            def pv_allowed(j, i):
                if j > i - SKEW:
                    return False
                if j < KC:
                    return j in vproj_popped and i >= vproj_popped[j] + 1
                return True

            pv_next = 0
            i = 0
            while pv_next < len(steps):
                if i < len(steps):
                    sc_stream(i)
                for _ in range(2):  # catch-up: up to two PV steps per loop
                    if pv_next < len(steps) and pv_allowed(pv_next, i):
                        pv_stream(pv_next)
                        pv_next += 1
                i += 1

            # tail: last norm, then the second o-proj batch
            for u in tail_norm:
                u()
            for qb in range(8, 16):
                oproj_unit(qb, tail=(qb % 2 == 0))()

            if debug:
                nc.sync.dma_start(dbg_qd[:], qd[0][:])
                nc.sync.dma_start(dbg_kd[:], kd[0][:])
                nc.sync.dma_start(dbg_xhat[:], xhat[:])
                nc.sync.dma_start(dbg_vsb[:], v_sb[:])

    nc.compile()
    return nc


def _get_nc(S):
    if S not in _NC_CACHE:
        _NC_CACHE[S] = _build_nc(S)
    return _NC_CACHE[S]


def kernel(query, key, value, mask, Wq, bq, Wk, bk, Wv, bv, Wo, bo):
    global LAST_RESULTS
    trace = os.environ.get("MHA_TRACE", "0") == "1"
    if trace:
        _register_ntff_hook()

    from concourse.bass_utils import run_bass_kernel_spmd

    query = np.asarray(query)
    key = np.asarray(key)
    value = np.asarray(value)
    mask = np.asarray(mask)
    Wq, bq, Wk, bk = map(np.asarray, (Wq, bq, Wk, bk))
    Wv, bv, Wo, bo = map(np.asarray, (Wv, bv, Wo, bo))

    S = query.shape[1]
    nc = _get_nc(S)

    bf = ml_dtypes.bfloat16
    maskTb = np.ascontiguousarray((mask[0] != 0).T).astype(bf)
    xT = {}
    for b in range(B):
        xT[("q", b)] = np.ascontiguousarray(query[b].T).astype(bf)
        xT[("k", b)] = np.ascontiguousarray(key[b].T).astype(bf)
        xT[("v", b)] = np.ascontiguousarray(value[b].T).astype(bf)

    def w_chunks(Wsl):
        # [128 out, 512 e] -> [128 p(e%128), 512 (ec*128 + out)]
        return np.ascontiguousarray(
            Wsl.T.reshape(4, P, P).transpose(1, 0, 2).reshape(P, D))

    in_maps = []
    for c in range(N_CORES):
        b, hp = divmod(c, 4)
        sl = slice(P * hp, P * hp + P)
        def dup(Wsl, h):
            return w_chunks(np.concatenate([Wsl[h * 64:(h + 1) * 64]] * 2, 0))

        wv_c = w_chunks(Wv[sl, :])
        wo_c = np.ascontiguousarray(Wo[:, sl].T)
        wcat = np.concatenate(
            [dup(Wq[sl], 0), dup(Wq[sl], 1), dup(Wk[sl], 0), dup(Wk[sl], 1),
             wv_c, wo_c], axis=1).astype(bf)

        def dupb(b, h):
            return np.concatenate([b[h * 64:(h + 1) * 64]] * 2, 0)

        bqk = np.stack([dupb(bq[sl], 0) / 16.0, dupb(bq[sl], 1) / 16.0,
                        dupb(bk[sl], 0), dupb(bk[sl], 1)],
                       axis=1).astype(np.float32)
        in_maps.append({
            "xqT": xT[("q", b)],
            "xkT": xT[("k", b)],
            "xvT": xT[("v", b)],
            "maskT": maskTb,
            "wcat": wcat,
            "bqk": bqk,
            "bv_row": bv[sl].reshape(1, P).astype(bf),
        })

    res = run_bass_kernel_spmd(
        nc, in_maps, core_ids=list(range(N_CORES)),
        trace=trace, trace_cores=[0] if trace else None,
    )
    LAST_RESULTS = res

    out = np.zeros((B, S, D), np.float32)
    for c in range(N_CORES):
        out[c // 4] += res.results[c]["outp"].astype(np.float32)
    out += bo.astype(np.float32)
    return out
